# revision 27
# baseline (speedup 1.0000x reference)
"""Trainium2 Bass kernel for nn_AttentionTD (3-block deformable attention TD).

Self-contained: hardcodes all shapes. Data-parallel over batch B=8 across the
8 NeuronCores; each core runs the full 3-block DAT stack for one batch element.

v2: software-pipelined blocks (prefix of block b+1 overlaps attention of block
b), native hw Gelu, c-batched index math, bf16 q projection, halved IO.
"""

import sys

sys.path.insert(0, "/opt/trn_rl_repo")

import numpy as np

# ---------------- problem constants ----------------
B, C, H, W = 8, 128, 64, 64
NCH = 64          # channels per DAT block
NH, HC = 4, 16    # heads, head channels
KS = 4
HWS = H * W       # 4096
HK = WK = 16
NS = HK * WK      # 256 sample points
EPS = 1e-5
NBLK = 3
# rpe slice table geometry: [blk][h][x0 (64)][pair-row (128)][2 (W,D)][col (65)]
TROW, TCOL = 128, 65
TSLICE = TROW * 2 * TCOL      # 16640
THEAD = 64 * TSLICE           # per (blk,h)
TBLK = NH * THEAD
NTAB = NBLK * TBLK

_CACHE = {}


def _build_graph():
    from concourse import bacc, mybir, tile
    import concourse.bass as bass
    from concourse.bass import IndirectOffsetOnAxis

    f32 = mybir.dt.float32
    bf16 = mybir.dt.bfloat16
    i32 = mybir.dt.int32
    Alu = mybir.AluOpType
    Act = mybir.ActivationFunctionType

    nc = bacc.Bacc("TRN2", target_bir_lowering=False, debug=False, num_devices=8)

    # ---- dram io ----
    # xq_j: bf16 query-half (rows 0:64) of the block input; xr_j: f32
    # residual-half (rows 64:128).  o_j = updated residual half only.
    xq1_d = nc.dram_tensor("xq1", [64, HWS], bf16, kind="ExternalInput").ap()
    xq2_d = nc.dram_tensor("xq2", [64, HWS], bf16, kind="ExternalInput").ap()
    kvT0_d = nc.dram_tensor("kvT0", [HWS, NCH], f32, kind="ExternalInput").ap()
    kvT1_d = nc.dram_tensor("kvT1", [HWS, NCH], f32, kind="ExternalInput").ap()
    wq_d = nc.dram_tensor("wq", [65, 3 * 128], bf16, kind="ExternalInput").ap()
    wpb_d = nc.dram_tensor("wpb", [65, 3 * 192], bf16, kind="ExternalInput").ap()
    cp_d = nc.dram_tensor("cp", [128, 590], f32, kind="ExternalInput").ap()
    cpb_d = nc.dram_tensor("cpb", [128, 320], bf16, kind="ExternalInput").ap()
    tab_d = nc.dram_tensor("rpetab", [NTAB, 1], bf16, kind="ExternalInput").ap()
    o1_d = nc.dram_tensor("o1", [64, HWS], bf16, kind="ExternalOutput").ap()
    o2_d = nc.dram_tensor("o2", [64, HWS], bf16, kind="ExternalOutput").ap()

    with tile.TileContext(nc) as tc:
        import contextlib

        ctx = contextlib.ExitStack()
        with ctx:
            cpool = ctx.enter_context(tc.tile_pool(name="const", bufs=1))
            qpool = ctx.enter_context(tc.tile_pool(name="qtiles", bufs=3))
            sb = ctx.enter_context(tc.tile_pool(name="work", bufs=1))
            sbs = ctx.enter_context(tc.tile_pool(name="small", bufs=2))
            bpool = ctx.enter_context(tc.tile_pool(name="blkstate", bufs=3))
            spool = ctx.enter_context(tc.tile_pool(name="slab", bufs=3))
            ypool = ctx.enter_context(tc.tile_pool(name="ytil", bufs=3))
            ppool = ctx.enter_context(tc.tile_pool(name="probs", bufs=1))
            apool = ctx.enter_context(tc.tile_pool(name="avs", bufs=1))
            qkps = ctx.enter_context(tc.tile_pool(name="qk", bufs=3, space="PSUM"))
            avps = ctx.enter_context(tc.tile_pool(name="av", bufs=2, space="PSUM"))
            mps = ctx.enter_context(tc.tile_pool(name="misc", bufs=1, space="PSUM"))
            tps = ctx.enter_context(tc.tile_pool(name="tailp", bufs=2, space="PSUM"))

            # ---- persistent loads ----
            cp = cpool.tile([128, 590], f32, tag="cp")
            nc.sync.dma_start(out=cp[:, :], in_=cp_d)
            wq = cpool.tile([65, 3 * 128], bf16, tag="wq")
            nc.sync.dma_start(out=wq[:, :], in_=wq_d)
            wpb = cpool.tile([65, 3 * 192], bf16, tag="wpb")
            nc.sync.dma_start(out=wpb[:, :], in_=wpb_d)
            cpb = cpool.tile([128, 320], bf16, tag="cpb")
            nc.sync.dma_start(out=cpb[:, :], in_=cpb_d)
            xq1 = cpool.tile([65, HWS], bf16, tag="xq1")
            nc.sync.dma_start(out=xq1[0:64, :], in_=xq1_d)
            nc.vector.memset(xq1[64:65, :], 1.0)
            xq2 = cpool.tile([65, HWS], bf16, tag="xq2")
            nc.sync.dma_start(out=xq2[0:64, :], in_=xq2_d)
            nc.vector.memset(xq2[64:65, :], 1.0)
            xo1 = cpool.tile([64, HWS], bf16, tag="xo1")
            xo2 = cpool.tile([64, HWS], bf16, tag="xo2")

            def act_raw(out, in_, func, eng=None):
                eng = eng or nc.scalar
                ins = [eng.lower_ap(in_)]
                for v in (0.0, 1.0, 0.0):
                    ins.append(mybir.ImmediateValue(dtype=mybir.dt.float32, value=v))
                return eng.add_instruction(
                    mybir.InstActivation(
                        name=nc.get_next_instruction_name(), func=func,
                        ins=ins, outs=[eng.lower_ap(out)],
                    )
                )

            zb = cpool.tile([128, 1], f32, tag="zb")
            nc.vector.memset(zb[:, :], 0.0)
            epst = cpool.tile([1, 1], f32, tag="epst")
            nc.vector.memset(epst[:, :], EPS)

            eye = cp[:, 0:128]
            ref_yx = cp[0:2, 128:384]          # row0 = y, row1 = x
            ones1_128 = cp[0:1, 384:512]       # [1,128] ones (bcast lhsT)
            ones128_div = cp[0:128, 520:521]   # 1/64 on data rows, 0 on gaps

            BI = {0: (xq1, kvT0_d, xo1), 1: (xq2, kvT0_d, xo2), 2: (xq2, kvT1_d, xo2)}
            ST = [dict() for _ in range(NBLK)]   # per-block live tiles

            # ---------------- prefix stage 1: q projection ----------------
            def pre_q(blk):
                s = ST[blk]
                XQ, _, _ = BI[blk]
                pq_wT_sp = wq[:, blk * 128: (blk + 1) * 128]
                q_b = qpool.tile([128, HWS], bf16, tag="qb")
                for mc in range(8):
                    qp = tps.tile([128, 512], f32, tag="tl")
                    nc.tensor.matmul(
                        out=qp[:, :], lhsT=pq_wT_sp, rhs=XQ[:, mc * 512:(mc + 1) * 512],
                        start=True, stop=True,
                    )
                    act_raw(q_b[:, mc * 512:(mc + 1) * 512], qp[:, :], Act.Copy)
                s["q_b"] = q_b

            # ------------- prefix stage 2: offsets / indices ---------------
            def pre_off(blk):
                s = ST[blk]
                q_b = s["q_b"]
                bc0 = 527 + blk * 21
                dw_w = cp[:, bc0: bc0 + 16]
                dw_b = cp[:, bc0 + 16: bc0 + 17]
                ln_g = cp[:, bc0 + 17: bc0 + 18]
                ln_b = cp[:, bc0 + 18: bc0 + 19]
                pw_wT = cp[:, bc0 + 19: bc0 + 21]

                # depthwise 4x4 stride-4 conv (on bf16 q)
                q5 = q_b[:, :].rearrange("p (hh a ww b) -> p hh a ww b", hh=16, a=4, ww=16, b=4)
                asq = sbs.tile([128, 2, NS], f32, tag="asq")
                acc = asq[:, 0, :]
                sq = asq[:, 1, :]
                nc.vector.tensor_scalar(
                    out=acc, in0=q5[:, :, 0, :, 0], scalar1=dw_w[:, 0:1],
                    scalar2=None, op0=Alu.mult,
                )
                for t in range(1, 16):
                    dy, dx = t // 4, t % 4
                    nc.vector.scalar_tensor_tensor(
                        out=acc, in0=q5[:, :, dy, :, dx],
                        scalar=dw_w[:, t: t + 1], in1=acc,
                        op0=Alu.mult, op1=Alu.add,
                    )
                nc.vector.tensor_scalar(
                    out=acc, in0=acc, scalar1=dw_b, scalar2=None, op0=Alu.add
                )

                # layernorm over channels (mean via matmul with 1/64 weights)
                nc.vector.tensor_tensor(out=sq, in0=acc, in1=acc, op=Alu.mult)
                me_p = mps.tile([1, 2 * NS], f32, tag="m")
                nc.tensor.matmul(out=me_p[:, :], lhsT=ones128_div, rhs=asq[:, :, :],
                                 start=True, stop=True)
                stats = sbs.tile([1, 2 * NS], f32, tag="stats")
                nc.vector.tensor_copy(out=stats[:, 0:NS], in_=me_p[:, 0:NS])
                mu2 = sbs.tile([1, NS], f32, tag="mu2")
                nc.vector.tensor_tensor(out=mu2[:, :], in0=stats[:, 0:NS], in1=stats[:, 0:NS], op=Alu.mult)
                var = sbs.tile([1, NS], f32, tag="var")
                nc.vector.tensor_tensor(out=var[:, :], in0=me_p[:, NS:2 * NS], in1=mu2[:, :], op=Alu.subtract)
                sd = sbs.tile([1, NS], f32, tag="sd")
                nc.scalar.activation(out=sd[:, :], in_=var[:, :], func=Act.Sqrt, bias=epst[:, :])
                nc.vector.reciprocal(out=stats[:, NS:2 * NS], in_=sd[:, :])
                bc_p = mps.tile([128, 2 * NS], f32, tag="m")
                nc.tensor.matmul(out=bc_p[:, :], lhsT=ones1_128, rhs=stats[:, :], start=True, stop=True)
                t1 = sbs.tile([128, NS], f32, tag="t1")
                nc.vector.tensor_tensor(out=t1[:, :], in0=acc, in1=bc_p[:, 0:NS], op=Alu.subtract)
                nc.vector.tensor_tensor(out=t1[:, :], in0=t1[:, :], in1=bc_p[:, NS:2 * NS], op=Alu.mult)
                nc.vector.tensor_scalar(
                    out=t1[:, :], in0=t1[:, :], scalar1=ln_g, scalar2=ln_b,
                    op0=Alu.mult, op1=Alu.add,
                )
                gl = sbs.tile([128, NS], f32, tag="gl")
                nc.scalar.activation(out=gl[:, :], in_=t1[:, :], func=Act.Gelu, bias=zb[:, :])

                # offsets -> positions
                off_p = mps.tile([2, NS], f32, tag="m")
                nc.tensor.matmul(out=off_p[:, :], lhsT=pw_wT, rhs=gl[:, :], start=True, stop=True)
                pos = sbs.tile([2, NS], f32, tag="pos")
                nc.vector.tensor_tensor(out=pos[:, :], in0=off_p[:, :], in1=ref_yx, op=Alu.add)
                nc.vector.tensor_scalar(
                    out=pos[:, :], in0=pos[:, :], scalar1=1.0, scalar2=-1.0,
                    op0=Alu.min, op1=Alu.max,
                )

                # transpose pos -> [n,(y,x)] per 128-chunk
                posT = sbs.tile([128, 4], f32, tag="posT")  # cols: c0y c0x c1y c1x
                for c in range(2):
                    tp = mps.tile([128, 2], f32, tag="m")
                    nc.tensor.transpose(
                        out=tp[:, :], in_=pos[:, c * 128:(c + 1) * 128], identity=eye[0:2, 0:2]
                    )
                    nc.vector.tensor_copy(out=posT[:, c * 2: c * 2 + 2], in_=tp[:, :])

                # ---- index & weight math, batched over the two 128-chunks ----
                p2 = posT[:, :].rearrange("p (c two) -> p two c", two=2)
                y = p2[:, 0, :]   # [128, 2] strided
                x = p2[:, 1, :]
                idxkv = sbs.tile([128, 4], f32, tag="idxkv")
                idxw = sbs.tile([128, 8], f32, tag="idxw")
                fyb = bpool.tile([128, 2], f32, tag="fyb")
                wkv = sbs.tile([128, 8], f32, tag="wkv")   # w00 w01 w10 w11 per chunk
                dxw = sbs.tile([128, 4], f32, tag="dxw")   # (1-fxb, fxb) per chunk
                scr = sbs.tile([128, 24], f32, tag="scr")
                s2 = scr[:, :].rearrange("p (k c) -> p k c", c=2)
                xf, yf = s2[:, 0, :], s2[:, 1, :]
                xm, ym = s2[:, 2, :], s2[:, 3, :]
                x0, y0 = s2[:, 4, :], s2[:, 5, :]
                fx, fy = s2[:, 6, :], s2[:, 7, :]
                fx1, fy1 = s2[:, 8, :], s2[:, 9, :]
                ib, iw = s2[:, 10, :], s2[:, 11, :]

                # kv pixel coords
                nc.vector.tensor_scalar(out=xf, in0=x, scalar1=1.0, scalar2=31.5, op0=Alu.add, op1=Alu.mult)
                nc.vector.tensor_scalar(out=yf, in0=y, scalar1=1.0, scalar2=31.5, op0=Alu.add, op1=Alu.mult)
                # floor via round-to-nearest (+2^23) then fix-up (r > x)
                nc.vector.tensor_scalar(out=x0, in0=xf, scalar1=8388608.0, scalar2=-8388608.0, op0=Alu.add, op1=Alu.add)
                nc.vector.tensor_tensor(out=xm, in0=x0, in1=xf, op=Alu.is_gt)
                nc.vector.tensor_tensor(out=x0, in0=x0, in1=xm, op=Alu.subtract)
                nc.vector.tensor_scalar(out=x0, in0=x0, scalar1=62.0, scalar2=None, op0=Alu.min)
                nc.vector.tensor_scalar(out=y0, in0=yf, scalar1=8388608.0, scalar2=-8388608.0, op0=Alu.add, op1=Alu.add)
                nc.vector.tensor_tensor(out=ym, in0=y0, in1=yf, op=Alu.is_gt)
                nc.vector.tensor_tensor(out=y0, in0=y0, in1=ym, op=Alu.subtract)
                nc.vector.tensor_scalar(out=y0, in0=y0, scalar1=62.0, scalar2=None, op0=Alu.min)
                nc.vector.tensor_tensor(out=fx, in0=xf, in1=x0, op=Alu.subtract)
                nc.vector.tensor_tensor(out=fy, in0=yf, in1=y0, op=Alu.subtract)
                nc.vector.tensor_scalar(out=fx1, in0=fx, scalar1=-1.0, scalar2=1.0, op0=Alu.mult, op1=Alu.add)
                nc.vector.tensor_scalar(out=fy1, in0=fy, scalar1=-1.0, scalar2=1.0, op0=Alu.mult, op1=Alu.add)
                w4 = wkv[:, :].rearrange("p (c t) -> p t c", t=4)
                nc.vector.tensor_tensor(out=w4[:, 0, :], in0=fy1, in1=fx1, op=Alu.mult)
                nc.vector.tensor_tensor(out=w4[:, 1, :], in0=fy1, in1=fx, op=Alu.mult)
                nc.vector.tensor_tensor(out=w4[:, 2, :], in0=fy, in1=fx1, op=Alu.mult)
                nc.vector.tensor_tensor(out=w4[:, 3, :], in0=fy, in1=fx, op=Alu.mult)
                # kv gather indices: y0*64+x0 (+0,+1,+64,+65)
                nc.vector.scalar_tensor_tensor(out=ib, in0=y0, scalar=64.0, in1=x0, op0=Alu.mult, op1=Alu.add)
                i4 = idxkv[:, :].rearrange("p (c t) -> p t c", t=2)
                for t, offt in enumerate((0.0, 64.0)):
                    nc.vector.tensor_scalar(
                        out=i4[:, t, :], in0=ib, scalar1=offt, scalar2=None, op0=Alu.add,
                    )
                # bias window coords: cx = 31.5*(1-x), cy = 31.5*(1-y)
                nc.vector.tensor_scalar(out=xf, in0=x, scalar1=-31.5, scalar2=31.5, op0=Alu.mult, op1=Alu.add)
                nc.vector.tensor_scalar(out=yf, in0=y, scalar1=-31.5, scalar2=31.5, op0=Alu.mult, op1=Alu.add)
                nc.vector.tensor_scalar(out=x0, in0=xf, scalar1=8388608.0, scalar2=-8388608.0, op0=Alu.add, op1=Alu.add)
                nc.vector.tensor_tensor(out=xm, in0=x0, in1=xf, op=Alu.is_gt)
                nc.vector.tensor_tensor(out=x0, in0=x0, in1=xm, op=Alu.subtract)
                nc.vector.tensor_scalar(out=y0, in0=yf, scalar1=8388608.0, scalar2=-8388608.0, op0=Alu.add, op1=Alu.add)
                nc.vector.tensor_tensor(out=ym, in0=y0, in1=yf, op=Alu.is_gt)
                nc.vector.tensor_tensor(out=y0, in0=y0, in1=ym, op=Alu.subtract)
                nc.vector.tensor_tensor(out=fx, in0=xf, in1=x0, op=Alu.subtract)
                nc.vector.tensor_tensor(out=fyb[:, :], in0=yf, in1=y0, op=Alu.subtract)
                d2 = dxw[:, :].rearrange("p (c two) -> p two c", two=2)
                nc.vector.tensor_scalar(out=d2[:, 0, :], in0=fx, scalar1=-1.0, scalar2=1.0, op0=Alu.mult, op1=Alu.add)
                nc.vector.tensor_copy(out=d2[:, 1, :], in_=fx)
                # window index: ((x0b*128)+y0b)*65 + blk_base (+h stride)
                nc.vector.scalar_tensor_tensor(out=iw, in0=x0, scalar=128.0, in1=y0, op0=Alu.mult, op1=Alu.add)
                nc.vector.tensor_scalar(
                    out=iw, in0=iw, scalar1=130.0, scalar2=float(blk * TBLK),
                    op0=Alu.mult, op1=Alu.add,
                )
                iw4 = idxw[:, :].rearrange("p (c t) -> p t c", t=4)
                for hh in range(4):
                    nc.vector.tensor_scalar(
                        out=iw4[:, hh, :], in0=iw,
                        scalar1=float(hh * THEAD), scalar2=None, op0=Alu.add,
                    )

                idxkv_i = sbs.tile([128, 4], i32, tag="idxkvi")
                nc.vector.tensor_copy(out=idxkv_i[:, :], in_=idxkv[:, :])
                idxw_i = bpool.tile([128, 8], i32, tag="idxwi")
                nc.vector.tensor_copy(out=idxw_i[:, :], in_=idxw[:, :])

                # diag weight matrices for the two x-taps, per chunk
                diags = []
                for c in range(2):
                    d0 = bpool.tile([128, 128], bf16, tag=f"d0_{c}")
                    d1 = bpool.tile([128, 128], bf16, tag=f"d1_{c}")
                    nc.vector.tensor_scalar(out=d0[:, :], in0=eye, scalar1=dxw[:, c * 2: c * 2 + 1], scalar2=None, op0=Alu.mult)
                    nc.vector.tensor_scalar(out=d1[:, :], in0=eye, scalar1=dxw[:, c * 2 + 1: c * 2 + 2], scalar2=None, op0=Alu.mult)
                    diags.append((d0, d1))
                s.update(idxkv_i=idxkv_i, idxw_i=idxw_i, fyb=fyb, wkv=wkv, diags=diags)

            # ------------- prefix stage 3: kv gather + k/v proj -------------
            def pre_kv(blk):
                s = ST[blk]
                _, kvT_ap, _ = BI[blk]
                idxkv_i, wkv = s["idxkv_i"], s["wkv"]
                pk_wTs1 = wpb[0:65, blk * 192: blk * 192 + 128]
                pv_wT1 = wpb[0:65, blk * 192 + 128: blk * 192 + 192]

                G = sb.tile([128, 4, 128], f32, tag="G")
                for j in range(4):
                    nc.gpsimd.indirect_dma_start(
                        out=G[:, j, :], out_offset=None, in_=kvT_ap,
                        in_offset=IndirectOffsetOnAxis(ap=idxkv_i[:, j: j + 1], axis=0),
                    )
                xs_b = sbs.tile([65, NS], bf16, tag="xsb")
                nc.vector.memset(xs_b[64:65, :], 1.0)
                for c in range(2):
                    xsT = sbs.tile([128, 64], f32, tag="xsT")
                    nc.vector.tensor_scalar(
                        out=xsT[:, :], in0=G[:, c * 2, 0:64],
                        scalar1=wkv[:, c * 4: c * 4 + 1], scalar2=None, op0=Alu.mult,
                    )
                    for t in range(1, 4):
                        gsl = G[:, c * 2 + t // 2, (t % 2) * 64:(t % 2) * 64 + 64]
                        nc.vector.scalar_tensor_tensor(
                            out=xsT[:, :], in0=gsl,
                            scalar=wkv[:, c * 4 + t: c * 4 + t + 1], in1=xsT[:, :],
                            op0=Alu.mult, op1=Alu.add,
                        )
                    xs_p = mps.tile([64, 128], f32, tag="m")
                    nc.tensor.transpose(out=xs_p[:, :], in_=xsT[:, :], identity=eye)
                    nc.vector.tensor_copy(out=xs_b[0:64, c * 128:(c + 1) * 128], in_=xs_p[:, :])

                k_p = mps.tile([128, NS], f32, tag="m")
                nc.tensor.matmul(out=k_p[:, :], lhsT=pk_wTs1, rhs=xs_b[:, :], start=True, stop=True)
                k_b = bpool.tile([128, NS], bf16, tag="kb")
                nc.vector.tensor_copy(out=k_b[:, :], in_=k_p[:, :])

                vT1 = bpool.tile([128, 2, 128], bf16, tag="vT1")
                nc.vector.memset(vT1[:, :, :], 0.0)
                nc.vector.memset(vT1[:, :, :].rearrange("p c (h q) -> p c h q", q=32)[:, :, :, 16:17], 1.0)
                for c in range(2):
                    v_p = mps.tile([128, 64], f32, tag="m")
                    nc.tensor.matmul(
                        out=v_p[:, :], lhsT=xs_b[:, c * 128:(c + 1) * 128], rhs=pv_wT1,
                        start=True, stop=True,
                    )
                    vv = vT1[:, c, :].rearrange("p (h q) -> p h q", q=32)
                    nc.vector.tensor_copy(
                        out=vv[:, :, 0:16],
                        in_=v_p[:, :].rearrange("p (h q) -> p h q", q=16),
                    )
                s.update(k_b=k_b, vT1=vT1)

            # ------------- prefix stage 4: window gathers (prefetch) --------
            def pre_win(blk, heads):
                s = ST[blk]
                idxw_i = s["idxw_i"]
                slabs = s.setdefault("slabs", {})
                for h in heads:
                    for c in range(2):
                        Sl = spool.tile([128, 8320], bf16, tag="S")
                        nc.gpsimd.indirect_dma_start(
                            out=Sl[:, :], out_offset=None, in_=tab_d,
                            in_offset=IndirectOffsetOnAxis(ap=idxw_i[:, c * 4 + h: c * 4 + h + 1], axis=0),
                        )
                        slabs[(h, c)] = Sl

            # ------- y-lerp blend for one head (emitted one head ahead) ------
            def blend(blk, h):
                s = ST[blk]
                fyb = s["fyb"]
                ys = s.setdefault("Y", {})
                for c in range(2):
                    Sl = s["slabs"].pop((h, c))
                    S3 = Sl[:, :].rearrange("p (r two q) -> p r two q", two=2, q=65)
                    Yt = ypool.tile([128, 4160], bf16, tag="Y")
                    Y2 = Yt[:, :].rearrange("p (r q) -> p r q", q=65)
                    nc.vector.scalar_tensor_tensor(
                        out=Y2[:, :, :], in0=S3[:, :, 1, :], scalar=fyb[:, c: c + 1],
                        in1=S3[:, :, 0, :], op0=Alu.mult, op1=Alu.add,
                    )
                    ys[(h, c)] = Yt

            # ---------------- attention for one head ----------------
            def attn_head(blk, h):
                s = ST[blk]
                q_b, k_b, vT1, diags = s["q_b"], s["k_b"], s["vT1"], s["diags"]
                if h == 0:
                    avs_t = apool.tile([128, HWS], bf16, tag="avs")
                    s["avs"] = avs_t
                # emit next head's blend first so DVE runs it under our matmuls
                if h < NH - 1:
                    blend(blk, h + 1)
                elif blk + 1 < NBLK:
                    blend(blk + 1, 0)
                avs = s["avs"]
                P = ppool.tile([128, 2, HWS], bf16, tag="P")
                for c in range(2):
                    Yt = s["Y"].pop((h, c))
                    Y3 = Yt[:, :].rearrange("p (r q) -> p r q", q=65)
                    d0, d1 = diags[c]
                    kh = k_b[h * 32: h * 32 + 16, c * 128:(c + 1) * 128]
                    for wv in range(4):
                        pts = []
                        for i in range(2):
                            mc = wv * 2 + i
                            pt = qkps.tile([128, 512], f32, tag="qkp")
                            nc.tensor.matmul(
                                out=pt[:, :], lhsT=kh,
                                rhs=q_b[h * 32: h * 32 + 16, mc * 512:(mc + 1) * 512],
                                start=True, stop=False, tile_position=(h * 32, 0),
                            )
                            pts.append(pt)
                        for i in range(2):
                            mc = wv * 2 + i
                            nc.tensor.matmul(
                                out=pts[i][:, :], lhsT=d0,
                                rhs=Y3[:, mc * 8:(mc + 1) * 8, 0:64],
                                start=False, stop=False,
                            )
                        for i in range(2):
                            mc = wv * 2 + i
                            nc.tensor.matmul(
                                out=pts[i][:, :], lhsT=d1,
                                rhs=Y3[:, mc * 8:(mc + 1) * 8, 1:65],
                                start=False, stop=True,
                            )
                        for i in range(2):
                            mc = wv * 2 + i
                            nc.scalar.activation(
                                out=P[:, c, mc * 512:(mc + 1) * 512], in_=pts[i][:, :],
                                func=Act.Exp, bias=zb[:, :],
                            )
                # AV for this head: [32,512] psum (row 16 = sums), then spread copy
                for pr in range(4):
                    mca, mcb = pr * 2, pr * 2 + 1
                    a0 = avps.tile([32, 512], f32, tag="avp")
                    a1 = avps.tile([32, 512], f32, tag="avp")
                    for c in range(2):
                        lw = vT1[:, c, h * 32:(h + 1) * 32]
                        nc.tensor.matmul(out=a0[:, :], lhsT=lw, rhs=P[:, c, mca * 512:(mca + 1) * 512], start=(c == 0), stop=(c == 1))
                        nc.tensor.matmul(out=a1[:, :], lhsT=lw, rhs=P[:, c, mcb * 512:(mcb + 1) * 512], start=(c == 0), stop=(c == 1))
                    act_raw(avs[h * 32:(h + 1) * 32, mca * 512:(mca + 1) * 512], a0[:, :], Act.Copy)
                    nc.vector.tensor_copy(out=avs[h * 32:(h + 1) * 32, mcb * 512:(mcb + 1) * 512], in_=a1[:, :])

            # ---------------- tail: normalize + out proj + residual --------
            def tail(blk):
                s = ST[blk]
                avs = s["avs"]
                _, _, R = BI[blk]
                po_wT_sp = cpb[:, 128 + blk * 64: 128 + (blk + 1) * 64]
                b4 = cpb[:, 0:128]
                po_b = cp[0:64, 524 + blk: 525 + blk]
                onf = apool.tile([128, HWS], bf16, tag="onf")
                for mc in range(8):
                    sb_p = tps.tile([128, 512], f32, tag="tl")
                    nc.tensor.matmul(out=sb_p[:, :], lhsT=b4, rhs=avs[:, mc * 512:(mc + 1) * 512], start=True, stop=True)
                    rcp = sbs.tile([128, 512], f32, tag="rcp")
                    act_raw(rcp[:, :], sb_p[:, :], Act.Reciprocal)
                    nc.vector.tensor_tensor(out=onf[:, mc * 512:(mc + 1) * 512], in0=avs[:, mc * 512:(mc + 1) * 512], in1=rcp[:, :], op=Alu.mult)
                for mc in range(8):
                    op = mps.tile([64, 512], f32, tag="m")
                    nc.tensor.matmul(out=op[:, :], lhsT=po_wT_sp, rhs=onf[:, mc * 512:(mc + 1) * 512], start=True, stop=True)
                    if blk == 2:
                        nc.vector.scalar_tensor_tensor(
                            out=R[:, mc * 512:(mc + 1) * 512], in0=op[:, :], scalar=po_b,
                            in1=R[:, mc * 512:(mc + 1) * 512], op0=Alu.add, op1=Alu.add,
                        )
                    else:
                        nc.vector.tensor_scalar(
                            out=R[:, mc * 512:(mc + 1) * 512], in0=op[:, :],
                            scalar1=po_b, scalar2=None, op0=Alu.add,
                        )

            # =================== pipelined emission ===================
            pre_q(0)
            pre_q(1)
            pre_q(2)
            pre_off(0)
            pre_kv(0)
            pre_win(0, (0, 1))
            blend(0, 0)
            for blk in range(NBLK):
                nxt = blk + 1
                for h in range(NH):
                    if nxt < NBLK:
                        if h == 2:
                            pre_off(nxt)
                        elif h == 3:
                            pre_kv(nxt)
                            pre_win(nxt, (0, 1))
                    attn_head(blk, h)
                    if h == 0:
                        pre_win(blk, (2, 3))
                tail(blk)
                if blk == 0:
                    nc.sync.dma_start(out=o1_d, in_=xo1[:, :])
            nc.sync.dma_start(out=o2_d, in_=xo2[:, :])

    nc.compile()
    return nc


def _host_prep(inputs):
    """Build per-core in_maps. inputs: dict of full numpy arrays."""
    import ml_dtypes

    x0, x1, x2 = inputs["x0"], inputs["x1"], inputs["x2"]

    def spread_cols(m):
        # m: [64(in), 64(out)] -> [64(in), 128] with out col h*16+j at h*32+j
        out = np.zeros((m.shape[0], 128), m.dtype)
        for h in range(4):
            out[:, h * 32: h * 32 + 16] = m[:, h * 16: (h + 1) * 16]
        return out

    def spread_rows(v):
        # v: [64, k] -> [128, k] with row h*16+j at h*32+j
        out = np.zeros((128,) + v.shape[1:], v.dtype)
        for h in range(4):
            out[h * 32: h * 32 + 16] = v[h * 16: (h + 1) * 16]
        return out

    # q-projection weights bf16: [65, 3*128]  (spread pq_wT, bias row 64)
    wq = np.zeros((65, 3 * 128), ml_dtypes.bfloat16)
    for b in range(3):
        wq[0:64, b * 128: (b + 1) * 128] = spread_cols(inputs["pq_w"][b].T).astype(
            ml_dtypes.bfloat16
        )
        wq[64, b * 128: (b + 1) * 128] = spread_rows(
            inputs["pq_b"][b][:, None]
        )[:, 0].astype(ml_dtypes.bfloat16)
    wpb = np.zeros((65, 3 * 192), ml_dtypes.bfloat16)
    for b in range(3):
        o = b * 192
        pk = np.zeros((65, 128), np.float32)
        pk[0:64] = spread_cols(inputs["pk_w"][b].T * 0.25)
        for h in range(4):
            pk[64, h * 32: h * 32 + 16] = inputs["pk_b"][b][h * 16: (h + 1) * 16] * 0.25
        wpb[:, o: o + 128] = pk.astype(ml_dtypes.bfloat16)
        wpb[:64, o + 128: o + 192] = inputs["pv_w"][b].T.astype(ml_dtypes.bfloat16)
        wpb[64, o + 128: o + 192] = inputs["pv_b"][b].astype(ml_dtypes.bfloat16)
    # const pack [128, 590]
    cp = np.zeros((128, 590), np.float32)
    cp[:, 0:128] = np.eye(128, dtype=np.float32)
    ys = (np.linspace(0.5, HK - 0.5, HK) / (HK - 1.0)) * 2.0 - 1.0
    cp[0, 128:384] = np.repeat(ys, WK)         # y per n (i-major)
    cp[1, 128:384] = np.tile(ys, HK)           # x per n
    cp[0, 384:512] = 1.0                       # ones1_128
    for h in range(4):
        cp[h * 32: h * 32 + 16, 520] = 1.0 / 64.0
    for b in range(3):
        cp[:, 521 + b] = spread_rows(inputs["pq_b"][b][:, None])[:, 0]
        cp[64:128, 524 + b] = inputs["po_b"][b]
        cp[0:64, 524 + b] = inputs["po_b"][b]
        bc0 = 527 + b * 21
        cp[:, bc0: bc0 + 16] = spread_rows(inputs["dw_w"][b].reshape(64, 16))
        cp[:, bc0 + 16] = spread_rows(inputs["dw_b"][b][:, None])[:, 0]
        cp[:, bc0 + 17] = spread_rows(inputs["ln_g"][b][:, None])[:, 0]
        cp[:, bc0 + 18] = spread_rows(inputs["ln_b"][b][:, None])[:, 0]
        cp[:, bc0 + 19: bc0 + 21] = spread_rows(inputs["pw_w"][b].T)
    cpb = np.zeros((128, 320), ml_dtypes.bfloat16)
    b4 = np.zeros((128, 128), np.float32)
    for h in range(4):
        b4[h * 32 + 16, h * 32: (h + 1) * 32] = 1.0
    cpb[:, 0:128] = b4.astype(ml_dtypes.bfloat16)
    for b in range(3):
        poT = inputs["po_w"][b].T  # [c, o]
        for h in range(4):
            cpb[h * 32: h * 32 + 16, 128 + b * 64: 128 + (b + 1) * 64] = poT[
                h * 16: (h + 1) * 16
            ].astype(ml_dtypes.bfloat16)
    # rpe slice tables bf16, row-interleaved: [b,h,x0,row,(W,D),col]
    tab = np.zeros((NBLK, NH, 64, TROW, 2, TCOL), ml_dtypes.bfloat16)
    rpe = inputs["rpe"]
    for b in range(3):
        for h in range(4):
            pad = np.zeros((129, 128), np.float32)
            pad[0:127, 0:127] = rpe[b, h]
            dif = pad[1:129] - pad[0:128]
            for x0s in range(64):
                tab[b, h, x0s, :, 0, :] = pad[0:128, x0s: x0s + 65].astype(ml_dtypes.bfloat16)
                tab[b, h, x0s, :, 1, :] = dif[:, x0s: x0s + 65].astype(ml_dtypes.bfloat16)
    tab = tab.reshape(-1, 1)

    in_maps = []
    for bb in range(B):
        m = {
            "xq1": np.ascontiguousarray(x1[bb, :64].reshape(64, HWS)).astype(ml_dtypes.bfloat16),
            "xq2": np.ascontiguousarray(x2[bb, :64].reshape(64, HWS)).astype(ml_dtypes.bfloat16),
            "kvT0": np.ascontiguousarray(x0[bb, :64].reshape(64, HWS).T),
            "kvT1": np.ascontiguousarray(x1[bb, :64].reshape(64, HWS).T),
            "wq": wq,
            "wpb": wpb,
            "cp": cp,
            "cpb": cpb,
            "rpetab": tab,
        }
        in_maps.append(m)
    return in_maps


def kernel(**inputs):
    from concourse.bass_utils import run_bass_kernel_spmd

    if "nc" not in _CACHE:
        _CACHE["nc"] = _build_graph()
    nc = _CACHE["nc"]
    in_maps = _host_prep(inputs)
    res = run_bass_kernel_spmd(nc, in_maps, core_ids=list(range(8)))
    out = np.zeros((NBLK, B, C, H, W), np.float32)
    out[0] = inputs["x0"]
    out[1, :, :64] = inputs["x1"][:, :64]
    out[2, :, :64] = inputs["x2"][:, :64]
    for bb in range(B):
        out[1, bb, 64:] = inputs["x1"][bb, 64:] + res.results[bb]["o1"].astype(np.float32).reshape(64, H, W)
        out[2, bb, 64:] = inputs["x2"][bb, 64:] + res.results[bb]["o2"].astype(np.float32).reshape(64, H, W)
    return out


# revision 30
# speedup vs baseline: 1.1666x; 1.1666x over previous
"""Trainium2 Bass kernel for nn_AttentionTD (3-block deformable attention TD).

Self-contained: hardcodes all shapes. Data-parallel over batch B=8 across the
8 NeuronCores; each core runs the full 3-block DAT stack for one batch element.

v2: software-pipelined blocks (prefix of block b+1 overlaps attention of block
b), native hw Gelu, c-batched index math, bf16 q projection, halved IO.
"""

import sys

sys.path.insert(0, "/opt/trn_rl_repo")

import numpy as np

# ---------------- problem constants ----------------
B, C, H, W = 8, 128, 64, 64
NCH = 64          # channels per DAT block
NH, HC = 4, 16    # heads, head channels
KS = 4
HWS = H * W       # 4096
HK = WK = 16
NS = HK * WK      # 256 sample points
EPS = 1e-5
NBLK = 3
# rpe slice table geometry: [blk][h][x0 (64)][pair-row (128)][2 (W,D)][col (65)]
TROW, TCOL = 128, 65
TSLICE = TROW * 2 * TCOL      # 16640
THEAD = 64 * TSLICE           # per (blk,h)
TBLK = NH * THEAD
NTAB = NBLK * TBLK

_CACHE = {}


def _build_graph():
    from concourse import bacc, mybir, tile
    import concourse.bass as bass
    from concourse.bass import IndirectOffsetOnAxis

    f32 = mybir.dt.float32
    bf16 = mybir.dt.bfloat16
    i32 = mybir.dt.int32
    Alu = mybir.AluOpType
    Act = mybir.ActivationFunctionType

    nc = bacc.Bacc("TRN2", target_bir_lowering=False, debug=False, num_devices=8)

    # ---- dram io ----
    # xq_j: bf16 query-half (rows 0:64) of the block input; xr_j: f32
    # residual-half (rows 64:128).  o_j = updated residual half only.
    xq1_d = nc.dram_tensor("xq1", [64, HWS], bf16, kind="ExternalInput").ap()
    xq2_d = nc.dram_tensor("xq2", [64, HWS], bf16, kind="ExternalInput").ap()
    kvT0_d = nc.dram_tensor("kvT0", [HWS, NCH], f32, kind="ExternalInput").ap()
    kvT1_d = nc.dram_tensor("kvT1", [HWS, NCH], f32, kind="ExternalInput").ap()
    wq_d = nc.dram_tensor("wq", [65, 3 * 128], bf16, kind="ExternalInput").ap()
    wpb_d = nc.dram_tensor("wpb", [65, 3 * 192], bf16, kind="ExternalInput").ap()
    cp_d = nc.dram_tensor("cp", [128, 590], f32, kind="ExternalInput").ap()
    cpb_d = nc.dram_tensor("cpb", [128, 320], bf16, kind="ExternalInput").ap()
    tab_d = nc.dram_tensor("rpetab", [NTAB, 1], bf16, kind="ExternalInput").ap()
    o1_d = nc.dram_tensor("o1", [64, HWS], bf16, kind="ExternalOutput").ap()
    o2_d = nc.dram_tensor("o2", [64, HWS], bf16, kind="ExternalOutput").ap()

    with tile.TileContext(nc) as tc:
        import contextlib

        ctx = contextlib.ExitStack()
        with ctx:
            cpool = ctx.enter_context(tc.tile_pool(name="const", bufs=1))
            qpool = ctx.enter_context(tc.tile_pool(name="qtiles", bufs=3))
            sb = ctx.enter_context(tc.tile_pool(name="work", bufs=1))
            sbs = ctx.enter_context(tc.tile_pool(name="small", bufs=2))
            bpool = ctx.enter_context(tc.tile_pool(name="blkstate", bufs=3))
            spool = ctx.enter_context(tc.tile_pool(name="slab", bufs=3))
            ypool = ctx.enter_context(tc.tile_pool(name="ytil", bufs=3))
            ppool = ctx.enter_context(tc.tile_pool(name="probs", bufs=1))
            apool = ctx.enter_context(tc.tile_pool(name="avs", bufs=1))
            qkps = ctx.enter_context(tc.tile_pool(name="qk", bufs=3, space="PSUM"))
            avps = ctx.enter_context(tc.tile_pool(name="av", bufs=2, space="PSUM"))
            mps = ctx.enter_context(tc.tile_pool(name="misc", bufs=1, space="PSUM"))
            tps = ctx.enter_context(tc.tile_pool(name="tailp", bufs=2, space="PSUM"))

            # ---- persistent loads ----
            cp = cpool.tile([128, 590], f32, tag="cp")
            nc.sync.dma_start(out=cp[:, :], in_=cp_d)
            wq = cpool.tile([65, 3 * 128], bf16, tag="wq")
            nc.sync.dma_start(out=wq[:, :], in_=wq_d)
            wpb = cpool.tile([65, 3 * 192], bf16, tag="wpb")
            nc.sync.dma_start(out=wpb[:, :], in_=wpb_d)
            cpb = cpool.tile([128, 320], bf16, tag="cpb")
            nc.sync.dma_start(out=cpb[:, :], in_=cpb_d)
            xq1 = cpool.tile([65, HWS], bf16, tag="xq1")
            nc.sync.dma_start(out=xq1[0:64, :], in_=xq1_d)
            nc.vector.memset(xq1[64:65, :], 1.0)
            xq2 = cpool.tile([65, HWS], bf16, tag="xq2")
            nc.sync.dma_start(out=xq2[0:64, :], in_=xq2_d)
            nc.vector.memset(xq2[64:65, :], 1.0)
            xo1 = cpool.tile([64, HWS], bf16, tag="xo1")
            xo2 = cpool.tile([64, HWS], bf16, tag="xo2")

            def act_raw(out, in_, func, eng=None):
                eng = eng or nc.scalar
                ins = [eng.lower_ap(in_)]
                for v in (0.0, 1.0, 0.0):
                    ins.append(mybir.ImmediateValue(dtype=mybir.dt.float32, value=v))
                return eng.add_instruction(
                    mybir.InstActivation(
                        name=nc.get_next_instruction_name(), func=func,
                        ins=ins, outs=[eng.lower_ap(out)],
                    )
                )

            zb = cpool.tile([128, 1], f32, tag="zb")
            nc.vector.memset(zb[:, :], 0.0)
            epst = cpool.tile([1, 1], f32, tag="epst")
            nc.vector.memset(epst[:, :], EPS)

            eye = cp[:, 0:128]
            ref_yx = cp[0:2, 128:384]          # row0 = y, row1 = x
            ones1_128 = cp[0:1, 384:512]       # [1,128] ones (bcast lhsT)
            ones128_div = cp[0:128, 520:521]   # 1/64 on data rows, 0 on gaps

            BI = {0: (xq1, kvT0_d, xo1), 1: (xq2, kvT0_d, xo2), 2: (xq2, kvT1_d, xo2)}
            ST = [dict() for _ in range(NBLK)]   # per-block live tiles

            # ---------------- prefix stage 1: q projection ----------------
            def pre_q(blk):
                s = ST[blk]
                XQ, _, _ = BI[blk]
                pq_wT_sp = wq[:, blk * 128: (blk + 1) * 128]
                q_b = qpool.tile([128, HWS], bf16, tag="qb")
                for mc in range(8):
                    qp = tps.tile([128, 512], f32, tag="tl")
                    nc.tensor.matmul(
                        out=qp[:, :], lhsT=pq_wT_sp, rhs=XQ[:, mc * 512:(mc + 1) * 512],
                        start=True, stop=True,
                    )
                    act_raw(q_b[:, mc * 512:(mc + 1) * 512], qp[:, :], Act.Copy)
                s["q_b"] = q_b

            # ------------- prefix stage 2: offsets / indices ---------------
            def pre_off(blk):
                s = ST[blk]
                q_b = s["q_b"]
                bc0 = 527 + blk * 21
                dw_w = cp[:, bc0: bc0 + 16]
                dw_b = cp[:, bc0 + 16: bc0 + 17]
                ln_g = cp[:, bc0 + 17: bc0 + 18]
                ln_b = cp[:, bc0 + 18: bc0 + 19]
                pw_wT = cp[:, bc0 + 19: bc0 + 21]

                # depthwise 4x4 stride-4 conv (on bf16 q)
                q5 = q_b[:, :].rearrange("p (hh a ww b) -> p hh a ww b", hh=16, a=4, ww=16, b=4)
                asq = sbs.tile([128, 2, NS], f32, tag="asq")
                acc = asq[:, 0, :]
                sq = asq[:, 1, :]
                nc.vector.tensor_scalar(
                    out=acc, in0=q5[:, :, 0, :, 0], scalar1=dw_w[:, 0:1],
                    scalar2=None, op0=Alu.mult,
                )
                for t in range(1, 16):
                    dy, dx = t // 4, t % 4
                    nc.vector.scalar_tensor_tensor(
                        out=acc, in0=q5[:, :, dy, :, dx],
                        scalar=dw_w[:, t: t + 1], in1=acc,
                        op0=Alu.mult, op1=Alu.add,
                    )
                nc.vector.tensor_scalar(
                    out=acc, in0=acc, scalar1=dw_b, scalar2=None, op0=Alu.add
                )

                # layernorm over channels (mean via matmul with 1/64 weights)
                nc.vector.tensor_tensor(out=sq, in0=acc, in1=acc, op=Alu.mult)
                me_p = mps.tile([1, 2 * NS], f32, tag="m")
                nc.tensor.matmul(out=me_p[:, :], lhsT=ones128_div, rhs=asq[:, :, :],
                                 start=True, stop=True)
                stats = sbs.tile([1, 2 * NS], f32, tag="stats")
                nc.vector.tensor_copy(out=stats[:, 0:NS], in_=me_p[:, 0:NS])
                mu2 = sbs.tile([1, NS], f32, tag="mu2")
                nc.vector.tensor_tensor(out=mu2[:, :], in0=stats[:, 0:NS], in1=stats[:, 0:NS], op=Alu.mult)
                var = sbs.tile([1, NS], f32, tag="var")
                nc.vector.tensor_tensor(out=var[:, :], in0=me_p[:, NS:2 * NS], in1=mu2[:, :], op=Alu.subtract)
                sd = sbs.tile([1, NS], f32, tag="sd")
                nc.scalar.activation(out=sd[:, :], in_=var[:, :], func=Act.Sqrt, bias=epst[:, :])
                nc.vector.reciprocal(out=stats[:, NS:2 * NS], in_=sd[:, :])
                bc_p = mps.tile([128, 2 * NS], f32, tag="m")
                nc.tensor.matmul(out=bc_p[:, :], lhsT=ones1_128, rhs=stats[:, :], start=True, stop=True)
                t1 = sbs.tile([128, NS], f32, tag="t1")
                nc.vector.tensor_tensor(out=t1[:, :], in0=acc, in1=bc_p[:, 0:NS], op=Alu.subtract)
                nc.vector.tensor_tensor(out=t1[:, :], in0=t1[:, :], in1=bc_p[:, NS:2 * NS], op=Alu.mult)
                nc.vector.tensor_scalar(
                    out=t1[:, :], in0=t1[:, :], scalar1=ln_g, scalar2=ln_b,
                    op0=Alu.mult, op1=Alu.add,
                )
                gl = sbs.tile([128, NS], f32, tag="gl")
                nc.scalar.activation(out=gl[:, :], in_=t1[:, :], func=Act.Gelu, bias=zb[:, :])

                # offsets -> positions
                off_p = mps.tile([2, NS], f32, tag="m")
                nc.tensor.matmul(out=off_p[:, :], lhsT=pw_wT, rhs=gl[:, :], start=True, stop=True)
                pos = sbs.tile([2, NS], f32, tag="pos")
                nc.vector.tensor_tensor(out=pos[:, :], in0=off_p[:, :], in1=ref_yx, op=Alu.add)
                nc.vector.tensor_scalar(
                    out=pos[:, :], in0=pos[:, :], scalar1=1.0, scalar2=-1.0,
                    op0=Alu.min, op1=Alu.max,
                )

                # transpose pos -> [n,(y,x)] per 128-chunk
                posT = sbs.tile([128, 4], f32, tag="posT")  # cols: c0y c0x c1y c1x
                for c in range(2):
                    tp = mps.tile([128, 2], f32, tag="m")
                    nc.tensor.transpose(
                        out=tp[:, :], in_=pos[:, c * 128:(c + 1) * 128], identity=eye[0:2, 0:2]
                    )
                    nc.vector.tensor_copy(out=posT[:, c * 2: c * 2 + 2], in_=tp[:, :])

                # ---- index & weight math, batched over the two 128-chunks ----
                p2 = posT[:, :].rearrange("p (c two) -> p two c", two=2)
                y = p2[:, 0, :]   # [128, 2] strided
                x = p2[:, 1, :]
                idxkv = sbs.tile([128, 4], f32, tag="idxkv")
                idxw = sbs.tile([128, 8], f32, tag="idxw")
                fyb = bpool.tile([128, 2], f32, tag="fyb")
                wkv = sbs.tile([128, 8], f32, tag="wkv")   # w00 w01 w10 w11 per chunk
                dxw = sbs.tile([128, 4], f32, tag="dxw")   # (1-fxb, fxb) per chunk
                scr = sbs.tile([128, 24], f32, tag="scr")
                s2 = scr[:, :].rearrange("p (k c) -> p k c", c=2)
                xf, yf = s2[:, 0, :], s2[:, 1, :]
                xm, ym = s2[:, 2, :], s2[:, 3, :]
                x0, y0 = s2[:, 4, :], s2[:, 5, :]
                fx, fy = s2[:, 6, :], s2[:, 7, :]
                fx1, fy1 = s2[:, 8, :], s2[:, 9, :]
                ib, iw = s2[:, 10, :], s2[:, 11, :]

                # kv pixel coords
                nc.vector.tensor_scalar(out=xf, in0=x, scalar1=1.0, scalar2=31.5, op0=Alu.add, op1=Alu.mult)
                nc.vector.tensor_scalar(out=yf, in0=y, scalar1=1.0, scalar2=31.5, op0=Alu.add, op1=Alu.mult)
                # floor via round-to-nearest (+2^23) then fix-up (r > x)
                nc.vector.tensor_scalar(out=x0, in0=xf, scalar1=8388608.0, scalar2=-8388608.0, op0=Alu.add, op1=Alu.add)
                nc.vector.tensor_tensor(out=xm, in0=x0, in1=xf, op=Alu.is_gt)
                nc.vector.tensor_tensor(out=x0, in0=x0, in1=xm, op=Alu.subtract)
                nc.vector.tensor_scalar(out=x0, in0=x0, scalar1=62.0, scalar2=None, op0=Alu.min)
                nc.vector.tensor_scalar(out=y0, in0=yf, scalar1=8388608.0, scalar2=-8388608.0, op0=Alu.add, op1=Alu.add)
                nc.vector.tensor_tensor(out=ym, in0=y0, in1=yf, op=Alu.is_gt)
                nc.vector.tensor_tensor(out=y0, in0=y0, in1=ym, op=Alu.subtract)
                nc.vector.tensor_scalar(out=y0, in0=y0, scalar1=62.0, scalar2=None, op0=Alu.min)
                nc.vector.tensor_tensor(out=fx, in0=xf, in1=x0, op=Alu.subtract)
                nc.vector.tensor_tensor(out=fy, in0=yf, in1=y0, op=Alu.subtract)
                nc.vector.tensor_scalar(out=fx1, in0=fx, scalar1=-1.0, scalar2=1.0, op0=Alu.mult, op1=Alu.add)
                nc.vector.tensor_scalar(out=fy1, in0=fy, scalar1=-1.0, scalar2=1.0, op0=Alu.mult, op1=Alu.add)
                w4 = wkv[:, :].rearrange("p (c t) -> p t c", t=4)
                nc.vector.tensor_tensor(out=w4[:, 0, :], in0=fy1, in1=fx1, op=Alu.mult)
                nc.vector.tensor_tensor(out=w4[:, 1, :], in0=fy1, in1=fx, op=Alu.mult)
                nc.vector.tensor_tensor(out=w4[:, 2, :], in0=fy, in1=fx1, op=Alu.mult)
                nc.vector.tensor_tensor(out=w4[:, 3, :], in0=fy, in1=fx, op=Alu.mult)
                # kv gather indices: y0*64+x0 (+0,+1,+64,+65)
                nc.vector.scalar_tensor_tensor(out=ib, in0=y0, scalar=64.0, in1=x0, op0=Alu.mult, op1=Alu.add)
                i4 = idxkv[:, :].rearrange("p (c t) -> p t c", t=2)
                for t, offt in enumerate((0.0, 64.0)):
                    nc.vector.tensor_scalar(
                        out=i4[:, t, :], in0=ib, scalar1=offt, scalar2=None, op0=Alu.add,
                    )
                # bias window coords: cx = 31.5*(1-x), cy = 31.5*(1-y)
                nc.vector.tensor_scalar(out=xf, in0=x, scalar1=-31.5, scalar2=31.5, op0=Alu.mult, op1=Alu.add)
                nc.vector.tensor_scalar(out=yf, in0=y, scalar1=-31.5, scalar2=31.5, op0=Alu.mult, op1=Alu.add)
                nc.vector.tensor_scalar(out=x0, in0=xf, scalar1=8388608.0, scalar2=-8388608.0, op0=Alu.add, op1=Alu.add)
                nc.vector.tensor_tensor(out=xm, in0=x0, in1=xf, op=Alu.is_gt)
                nc.vector.tensor_tensor(out=x0, in0=x0, in1=xm, op=Alu.subtract)
                nc.vector.tensor_scalar(out=y0, in0=yf, scalar1=8388608.0, scalar2=-8388608.0, op0=Alu.add, op1=Alu.add)
                nc.vector.tensor_tensor(out=ym, in0=y0, in1=yf, op=Alu.is_gt)
                nc.vector.tensor_tensor(out=y0, in0=y0, in1=ym, op=Alu.subtract)
                nc.vector.tensor_tensor(out=fx, in0=xf, in1=x0, op=Alu.subtract)
                nc.vector.tensor_tensor(out=fyb[:, :], in0=yf, in1=y0, op=Alu.subtract)
                d2 = dxw[:, :].rearrange("p (c two) -> p two c", two=2)
                nc.vector.tensor_scalar(out=d2[:, 0, :], in0=fx, scalar1=-1.0, scalar2=1.0, op0=Alu.mult, op1=Alu.add)
                nc.vector.tensor_copy(out=d2[:, 1, :], in_=fx)
                # window index: ((x0b*128)+y0b)*65 + blk_base (+h stride)
                nc.vector.scalar_tensor_tensor(out=iw, in0=x0, scalar=128.0, in1=y0, op0=Alu.mult, op1=Alu.add)
                nc.vector.tensor_scalar(
                    out=iw, in0=iw, scalar1=130.0, scalar2=float(blk * TBLK),
                    op0=Alu.mult, op1=Alu.add,
                )
                iw4 = idxw[:, :].rearrange("p (c t) -> p t c", t=4)
                for hh in range(4):
                    nc.vector.tensor_scalar(
                        out=iw4[:, hh, :], in0=iw,
                        scalar1=float(hh * THEAD), scalar2=None, op0=Alu.add,
                    )

                idxkv_i = sbs.tile([128, 4], i32, tag="idxkvi")
                nc.vector.tensor_copy(out=idxkv_i[:, :], in_=idxkv[:, :])
                idxw_i = bpool.tile([128, 8], i32, tag="idxwi")
                nc.vector.tensor_copy(out=idxw_i[:, :], in_=idxw[:, :])

                # diag weight matrices for the two x-taps, per chunk
                diags = []
                for c in range(2):
                    d0 = bpool.tile([128, 128], bf16, tag=f"d0_{c}")
                    d1 = bpool.tile([128, 128], bf16, tag=f"d1_{c}")
                    nc.vector.tensor_scalar(out=d0[:, :], in0=eye, scalar1=dxw[:, c * 2: c * 2 + 1], scalar2=None, op0=Alu.mult)
                    nc.vector.tensor_scalar(out=d1[:, :], in0=eye, scalar1=dxw[:, c * 2 + 1: c * 2 + 2], scalar2=None, op0=Alu.mult)
                    diags.append((d0, d1))
                s.update(idxkv_i=idxkv_i, idxw_i=idxw_i, fyb=fyb, wkv=wkv, diags=diags)

            # ------------- prefix stage 3: kv gather + k/v proj -------------
            def pre_kv(blk):
                s = ST[blk]
                _, kvT_ap, _ = BI[blk]
                idxkv_i, wkv = s["idxkv_i"], s["wkv"]
                pk_wTs1 = wpb[0:65, blk * 192: blk * 192 + 128]
                pv_wT1 = wpb[0:65, blk * 192 + 128: blk * 192 + 192]

                G = sb.tile([128, 4, 128], f32, tag="G")
                for j in range(4):
                    nc.gpsimd.indirect_dma_start(
                        out=G[:, j, :], out_offset=None, in_=kvT_ap,
                        in_offset=IndirectOffsetOnAxis(ap=idxkv_i[:, j: j + 1], axis=0),
                    )
                xs_b = sbs.tile([65, NS], bf16, tag="xsb")
                nc.vector.memset(xs_b[64:65, :], 1.0)
                for c in range(2):
                    xsT = sbs.tile([128, 64], f32, tag="xsT")
                    nc.vector.tensor_scalar(
                        out=xsT[:, :], in0=G[:, c * 2, 0:64],
                        scalar1=wkv[:, c * 4: c * 4 + 1], scalar2=None, op0=Alu.mult,
                    )
                    for t in range(1, 4):
                        gsl = G[:, c * 2 + t // 2, (t % 2) * 64:(t % 2) * 64 + 64]
                        nc.vector.scalar_tensor_tensor(
                            out=xsT[:, :], in0=gsl,
                            scalar=wkv[:, c * 4 + t: c * 4 + t + 1], in1=xsT[:, :],
                            op0=Alu.mult, op1=Alu.add,
                        )
                    xs_p = mps.tile([64, 128], f32, tag="m")
                    nc.tensor.transpose(out=xs_p[:, :], in_=xsT[:, :], identity=eye)
                    nc.vector.tensor_copy(out=xs_b[0:64, c * 128:(c + 1) * 128], in_=xs_p[:, :])

                k_p = mps.tile([128, NS], f32, tag="m")
                nc.tensor.matmul(out=k_p[:, :], lhsT=pk_wTs1, rhs=xs_b[:, :], start=True, stop=True)
                k_b = bpool.tile([128, NS], bf16, tag="kb")
                nc.vector.tensor_copy(out=k_b[:, :], in_=k_p[:, :])

                vT1 = bpool.tile([128, 2, 128], bf16, tag="vT1")
                nc.vector.memset(vT1[:, :, :], 0.0)
                nc.vector.memset(vT1[:, :, :].rearrange("p c (h q) -> p c h q", q=32)[:, :, :, 16:17], 1.0)
                for c in range(2):
                    v_p = mps.tile([128, 64], f32, tag="m")
                    nc.tensor.matmul(
                        out=v_p[:, :], lhsT=xs_b[:, c * 128:(c + 1) * 128], rhs=pv_wT1,
                        start=True, stop=True,
                    )
                    vv = vT1[:, c, :].rearrange("p (h q) -> p h q", q=32)
                    nc.vector.tensor_copy(
                        out=vv[:, :, 0:16],
                        in_=v_p[:, :].rearrange("p (h q) -> p h q", q=16),
                    )
                s.update(k_b=k_b, vT1=vT1)

            # ------------- prefix stage 4: window gathers (prefetch) --------
            def pre_win(blk, heads):
                s = ST[blk]
                idxw_i = s["idxw_i"]
                slabs = s.setdefault("slabs", {})
                for h in heads:
                    for c in range(2):
                        Sl = spool.tile([128, 8320], bf16, tag="S")
                        nc.gpsimd.indirect_dma_start(
                            out=Sl[:, :], out_offset=None, in_=tab_d,
                            in_offset=IndirectOffsetOnAxis(ap=idxw_i[:, c * 4 + h: c * 4 + h + 1], axis=0),
                        )
                        slabs[(h, c)] = Sl

            # ------- y-lerp blend for one head (emitted one head ahead) ------
            def blend(blk, h):
                s = ST[blk]
                fyb = s["fyb"]
                ys = s.setdefault("Y", {})
                for c in range(2):
                    Sl = s["slabs"].pop((h, c))
                    S3 = Sl[:, :].rearrange("p (r two q) -> p r two q", two=2, q=65)
                    Yt = ypool.tile([128, 4160], bf16, tag="Y")
                    Y2 = Yt[:, :].rearrange("p (r q) -> p r q", q=65)
                    nc.vector.scalar_tensor_tensor(
                        out=Y2[:, :, :], in0=S3[:, :, 1, :], scalar=fyb[:, c: c + 1],
                        in1=S3[:, :, 0, :], op0=Alu.mult, op1=Alu.add,
                    )
                    ys[(h, c)] = Yt

            # ---------------- attention for one head ----------------
            def attn_head(blk, h):
                s = ST[blk]
                q_b, k_b, vT1, diags = s["q_b"], s["k_b"], s["vT1"], s["diags"]
                if h == 0:
                    avs_t = apool.tile([128, HWS], bf16, tag="avs")
                    s["avs"] = avs_t
                # emit next head's blend first so DVE runs it under our matmuls
                if h < NH - 1:
                    blend(blk, h + 1)
                elif blk + 1 < NBLK:
                    blend(blk + 1, 0)
                avs = s["avs"]
                P = ppool.tile([128, 2, HWS], bf16, tag="P")
                for c in range(2):
                    Yt = s["Y"].pop((h, c))
                    Y3 = Yt[:, :].rearrange("p (r q) -> p r q", q=65)
                    d0, d1 = diags[c]
                    kh = k_b[h * 32: h * 32 + 16, c * 128:(c + 1) * 128]
                    for wv in range(4):
                        pts = []
                        for i in range(2):
                            mc = wv * 2 + i
                            pt = qkps.tile([128, 512], f32, tag="qkp")
                            nc.tensor.matmul(
                                out=pt[:, :], lhsT=kh,
                                rhs=q_b[h * 32: h * 32 + 16, mc * 512:(mc + 1) * 512],
                                start=True, stop=False, tile_position=(h * 32, 0),
                            )
                            pts.append(pt)
                        for i in range(2):
                            mc = wv * 2 + i
                            nc.tensor.matmul(
                                out=pts[i][:, :], lhsT=d0,
                                rhs=Y3[:, mc * 8:(mc + 1) * 8, 0:64],
                                start=False, stop=False,
                            )
                        for i in range(2):
                            mc = wv * 2 + i
                            nc.tensor.matmul(
                                out=pts[i][:, :], lhsT=d1,
                                rhs=Y3[:, mc * 8:(mc + 1) * 8, 1:65],
                                start=False, stop=True,
                            )
                        for i in range(2):
                            mc = wv * 2 + i
                            nc.scalar.activation(
                                out=P[:, c, mc * 512:(mc + 1) * 512], in_=pts[i][:, :],
                                func=Act.Exp, bias=zb[:, :],
                            )
                # AV for this head: [32,512] psum (row 16 = sums), then spread copy
                for pr in range(4):
                    mca, mcb = pr * 2, pr * 2 + 1
                    a0 = avps.tile([32, 512], f32, tag="avp")
                    a1 = avps.tile([32, 512], f32, tag="avp")
                    for c in range(2):
                        lw = vT1[:, c, h * 32:(h + 1) * 32]
                        nc.tensor.matmul(out=a0[:, :], lhsT=lw, rhs=P[:, c, mca * 512:(mca + 1) * 512], start=(c == 0), stop=(c == 1))
                        nc.tensor.matmul(out=a1[:, :], lhsT=lw, rhs=P[:, c, mcb * 512:(mcb + 1) * 512], start=(c == 0), stop=(c == 1))
                    act_raw(avs[h * 32:(h + 1) * 32, mca * 512:(mca + 1) * 512], a0[:, :], Act.Copy)
                    nc.vector.tensor_copy(out=avs[h * 32:(h + 1) * 32, mcb * 512:(mcb + 1) * 512], in_=a1[:, :])

            # ---------------- tail: normalize + out proj + residual --------
            def tail(blk):
                s = ST[blk]
                avs = s["avs"]
                _, _, R = BI[blk]
                po_wT_sp = cpb[:, 128 + blk * 64: 128 + (blk + 1) * 64]
                b4 = cpb[:, 0:128]
                po_b = cp[0:64, 524 + blk: 525 + blk]
                onf = apool.tile([128, HWS], bf16, tag="onf")
                for mc in range(8):
                    sb_p = tps.tile([128, 512], f32, tag="tl")
                    nc.tensor.matmul(out=sb_p[:, :], lhsT=b4, rhs=avs[:, mc * 512:(mc + 1) * 512], start=True, stop=True)
                    rcp = sbs.tile([128, 512], f32, tag="rcp")
                    act_raw(rcp[:, :], sb_p[:, :], Act.Reciprocal)
                    nc.vector.tensor_tensor(out=onf[:, mc * 512:(mc + 1) * 512], in0=avs[:, mc * 512:(mc + 1) * 512], in1=rcp[:, :], op=Alu.mult)
                for mc in range(8):
                    op = mps.tile([64, 512], f32, tag="m")
                    nc.tensor.matmul(out=op[:, :], lhsT=po_wT_sp, rhs=onf[:, mc * 512:(mc + 1) * 512], start=True, stop=True)
                    if blk == 2:
                        nc.vector.scalar_tensor_tensor(
                            out=R[:, mc * 512:(mc + 1) * 512], in0=op[:, :], scalar=po_b,
                            in1=R[:, mc * 512:(mc + 1) * 512], op0=Alu.add, op1=Alu.add,
                        )
                    else:
                        nc.vector.tensor_scalar(
                            out=R[:, mc * 512:(mc + 1) * 512], in0=op[:, :],
                            scalar1=po_b, scalar2=None, op0=Alu.add,
                        )

            # =================== pipelined emission ===================
            pre_q(0)
            pre_q(1)
            pre_q(2)
            pre_off(0)
            pre_kv(0)
            pre_win(0, (0, 1))
            blend(0, 0)
            for blk in range(NBLK):
                nxt = blk + 1
                for h in range(NH):
                    if nxt < NBLK:
                        if h == 2:
                            pre_off(nxt)
                        elif h == 3:
                            pre_kv(nxt)
                            pre_win(nxt, (0, 1))
                    attn_head(blk, h)
                    if h == 0:
                        pre_win(blk, (2, 3))
                tail(blk)
                if blk == 0:
                    nc.sync.dma_start(out=o1_d, in_=xo1[:, :])
            nc.sync.dma_start(out=o2_d, in_=xo2[:, :])

    nc.compile()
    return nc


def _host_prep(inputs):
    """Build per-core in_maps. inputs: dict of full numpy arrays."""
    import ml_dtypes

    x0, x1, x2 = inputs["x0"], inputs["x1"], inputs["x2"]

    def spread_cols(m):
        # m: [64(in), 64(out)] -> [64(in), 128] with out col h*16+j at h*32+j
        out = np.zeros((m.shape[0], 128), m.dtype)
        for h in range(4):
            out[:, h * 32: h * 32 + 16] = m[:, h * 16: (h + 1) * 16]
        return out

    def spread_rows(v):
        # v: [64, k] -> [128, k] with row h*16+j at h*32+j
        out = np.zeros((128,) + v.shape[1:], v.dtype)
        for h in range(4):
            out[h * 32: h * 32 + 16] = v[h * 16: (h + 1) * 16]
        return out

    # q-projection weights bf16: [65, 3*128]  (spread pq_wT, bias row 64)
    wq = np.zeros((65, 3 * 128), ml_dtypes.bfloat16)
    for b in range(3):
        wq[0:64, b * 128: (b + 1) * 128] = spread_cols(inputs["pq_w"][b].T).astype(
            ml_dtypes.bfloat16
        )
        wq[64, b * 128: (b + 1) * 128] = spread_rows(
            inputs["pq_b"][b][:, None]
        )[:, 0].astype(ml_dtypes.bfloat16)
    wpb = np.zeros((65, 3 * 192), ml_dtypes.bfloat16)
    for b in range(3):
        o = b * 192
        pk = np.zeros((65, 128), np.float32)
        pk[0:64] = spread_cols(inputs["pk_w"][b].T * 0.25)
        for h in range(4):
            pk[64, h * 32: h * 32 + 16] = inputs["pk_b"][b][h * 16: (h + 1) * 16] * 0.25
        wpb[:, o: o + 128] = pk.astype(ml_dtypes.bfloat16)
        wpb[:64, o + 128: o + 192] = inputs["pv_w"][b].T.astype(ml_dtypes.bfloat16)
        wpb[64, o + 128: o + 192] = inputs["pv_b"][b].astype(ml_dtypes.bfloat16)
    # const pack [128, 590]
    cp = np.zeros((128, 590), np.float32)
    cp[:, 0:128] = np.eye(128, dtype=np.float32)
    ys = (np.linspace(0.5, HK - 0.5, HK) / (HK - 1.0)) * 2.0 - 1.0
    cp[0, 128:384] = np.repeat(ys, WK)         # y per n (i-major)
    cp[1, 128:384] = np.tile(ys, HK)           # x per n
    cp[0, 384:512] = 1.0                       # ones1_128
    for h in range(4):
        cp[h * 32: h * 32 + 16, 520] = 1.0 / 64.0
    for b in range(3):
        cp[:, 521 + b] = spread_rows(inputs["pq_b"][b][:, None])[:, 0]
        cp[64:128, 524 + b] = inputs["po_b"][b]
        cp[0:64, 524 + b] = inputs["po_b"][b]
        bc0 = 527 + b * 21
        cp[:, bc0: bc0 + 16] = spread_rows(inputs["dw_w"][b].reshape(64, 16))
        cp[:, bc0 + 16] = spread_rows(inputs["dw_b"][b][:, None])[:, 0]
        cp[:, bc0 + 17] = spread_rows(inputs["ln_g"][b][:, None])[:, 0]
        cp[:, bc0 + 18] = spread_rows(inputs["ln_b"][b][:, None])[:, 0]
        cp[:, bc0 + 19: bc0 + 21] = spread_rows(inputs["pw_w"][b].T)
    cpb = np.zeros((128, 320), ml_dtypes.bfloat16)
    b4 = np.zeros((128, 128), np.float32)
    for h in range(4):
        b4[h * 32 + 16, h * 32: (h + 1) * 32] = 1.0
    cpb[:, 0:128] = b4.astype(ml_dtypes.bfloat16)
    for b in range(3):
        poT = inputs["po_w"][b].T  # [c, o]
        for h in range(4):
            cpb[h * 32: h * 32 + 16, 128 + b * 64: 128 + (b + 1) * 64] = poT[
                h * 16: (h + 1) * 16
            ].astype(ml_dtypes.bfloat16)
    # rpe slice tables bf16, row-interleaved: [b,h,x0,row,(W,D),col]
    tab = np.zeros((NBLK, NH, 64, TROW, 2, TCOL), ml_dtypes.bfloat16)
    rpe = inputs["rpe"]
    for b in range(3):
        for h in range(4):
            pad = np.zeros((129, 128), np.float32)
            pad[0:127, 0:127] = rpe[b, h]
            dif = pad[1:129] - pad[0:128]
            for x0s in range(64):
                tab[b, h, x0s, :, 0, :] = pad[0:128, x0s: x0s + 65].astype(ml_dtypes.bfloat16)
                tab[b, h, x0s, :, 1, :] = dif[:, x0s: x0s + 65].astype(ml_dtypes.bfloat16)
    tab = tab.reshape(-1, 1)

    in_maps = []
    for bb in range(B):
        m = {
            "xq1": np.ascontiguousarray(x1[bb, :64].reshape(64, HWS)).astype(ml_dtypes.bfloat16),
            "xq2": np.ascontiguousarray(x2[bb, :64].reshape(64, HWS)).astype(ml_dtypes.bfloat16),
            "kvT0": np.ascontiguousarray(x0[bb, :64].reshape(64, HWS).T),
            "kvT1": np.ascontiguousarray(x1[bb, :64].reshape(64, HWS).T),
            "wq": wq,
            "wpb": wpb,
            "cp": cp,
            "cpb": cpb,
            "rpetab": tab,
        }
        in_maps.append(m)
    return in_maps


def kernel(**inputs):
    from concourse.bass_utils import run_bass_kernel_spmd

    if "nc" not in _CACHE:
        _CACHE["nc"] = _build_graph()
    nc = _CACHE["nc"]
    in_maps = _host_prep(inputs)
    res = run_bass_kernel_spmd(nc, in_maps, core_ids=list(range(8)))
    out = np.zeros((NBLK, B, C, H, W), np.float32)
    out[0] = inputs["x0"]
    out[1, :, :64] = inputs["x1"][:, :64]
    out[2, :, :64] = inputs["x2"][:, :64]
    for bb in range(B):
        out[1, bb, 64:] = inputs["x1"][bb, 64:] + res.results[bb]["o1"].astype(np.float32).reshape(64, H, W)
        out[2, bb, 64:] = inputs["x2"][bb, 64:] + res.results[bb]["o2"].astype(np.float32).reshape(64, H, W)
    return out


# revision 32
# speedup vs baseline: 1.1891x; 1.0193x over previous
"""Trainium2 Bass kernel for nn_AttentionTD (3-block deformable attention TD).

Self-contained: hardcodes all shapes. Data-parallel over batch B=8 across the
8 NeuronCores; each core runs the full 3-block DAT stack for one batch element.

v2: software-pipelined blocks (prefix of block b+1 overlaps attention of block
b), native hw Gelu, c-batched index math, bf16 q projection, halved IO.
"""

import sys

sys.path.insert(0, "/opt/trn_rl_repo")

import numpy as np

# ---------------- problem constants ----------------
B, C, H, W = 8, 128, 64, 64
NCH = 64          # channels per DAT block
NH, HC = 4, 16    # heads, head channels
KS = 4
HWS = H * W       # 4096
HK = WK = 16
NS = HK * WK      # 256 sample points
EPS = 1e-5
NBLK = 3
# rpe slice table geometry: [blk][h][x0 (64)][pair-row (128)][2 (W,D)][col (65)]
TROW, TCOL = 128, 65
TSLICE = TROW * 2 * TCOL      # 16640
THEAD = 64 * TSLICE           # per (blk,h)
TBLK = NH * THEAD
NTAB = NBLK * TBLK

_CACHE = {}


def _build_graph():
    from concourse import bacc, mybir, tile
    import concourse.bass as bass
    from concourse.bass import IndirectOffsetOnAxis

    f32 = mybir.dt.float32
    bf16 = mybir.dt.bfloat16
    i32 = mybir.dt.int32
    Alu = mybir.AluOpType
    Act = mybir.ActivationFunctionType

    nc = bacc.Bacc("TRN2", target_bir_lowering=False, debug=False, num_devices=8)

    # ---- dram io ----
    # xq_j: bf16 query-half (rows 0:64) of the block input; xr_j: f32
    # residual-half (rows 64:128).  o_j = updated residual half only.
    xq1_d = nc.dram_tensor("xq1", [64, HWS], bf16, kind="ExternalInput").ap()
    xq2_d = nc.dram_tensor("xq2", [64, HWS], bf16, kind="ExternalInput").ap()
    kvT0_d = nc.dram_tensor("kvT0", [HWS, NCH], f32, kind="ExternalInput").ap()
    kvT1_d = nc.dram_tensor("kvT1", [HWS, NCH], f32, kind="ExternalInput").ap()
    wq_d = nc.dram_tensor("wq", [65, 3 * 128], bf16, kind="ExternalInput").ap()
    wpb_d = nc.dram_tensor("wpb", [65, 3 * 192], bf16, kind="ExternalInput").ap()
    cp_d = nc.dram_tensor("cp", [128, 590], f32, kind="ExternalInput").ap()
    cpb_d = nc.dram_tensor("cpb", [128, 320], bf16, kind="ExternalInput").ap()
    tab_d = nc.dram_tensor("rpetab", [NTAB, 1], bf16, kind="ExternalInput").ap()
    o1_d = nc.dram_tensor("o1", [64, HWS], bf16, kind="ExternalOutput").ap()
    o2_d = nc.dram_tensor("o2", [64, HWS], bf16, kind="ExternalOutput").ap()

    with tile.TileContext(nc) as tc:
        import contextlib

        ctx = contextlib.ExitStack()
        with ctx:
            cpool = ctx.enter_context(tc.tile_pool(name="const", bufs=1))
            qpool = ctx.enter_context(tc.tile_pool(name="qtiles", bufs=3))
            sb = ctx.enter_context(tc.tile_pool(name="work", bufs=1))
            sbs = ctx.enter_context(tc.tile_pool(name="small", bufs=2))
            bpool = ctx.enter_context(tc.tile_pool(name="blkstate", bufs=3))
            spool = ctx.enter_context(tc.tile_pool(name="slab", bufs=3))
            ypool = ctx.enter_context(tc.tile_pool(name="ytil", bufs=3))
            ppool = ctx.enter_context(tc.tile_pool(name="probs", bufs=1))
            apool = ctx.enter_context(tc.tile_pool(name="avs", bufs=1))
            qkps = ctx.enter_context(tc.tile_pool(name="qk", bufs=3, space="PSUM"))
            avps = ctx.enter_context(tc.tile_pool(name="av", bufs=2, space="PSUM"))
            mps = ctx.enter_context(tc.tile_pool(name="misc", bufs=1, space="PSUM"))
            tps = ctx.enter_context(tc.tile_pool(name="tailp", bufs=2, space="PSUM"))

            # ---- persistent loads ----
            cp = cpool.tile([128, 590], f32, tag="cp")
            nc.sync.dma_start(out=cp[:, :], in_=cp_d)
            wq = cpool.tile([65, 3 * 128], bf16, tag="wq")
            nc.sync.dma_start(out=wq[:, :], in_=wq_d)
            wpb = cpool.tile([65, 3 * 192], bf16, tag="wpb")
            nc.sync.dma_start(out=wpb[:, :], in_=wpb_d)
            cpb = cpool.tile([128, 320], bf16, tag="cpb")
            nc.sync.dma_start(out=cpb[:, :], in_=cpb_d)
            xq1 = cpool.tile([65, HWS], bf16, tag="xq1")
            nc.sync.dma_start(out=xq1[0:64, :], in_=xq1_d)
            nc.vector.memset(xq1[64:65, :], 1.0)
            xq2 = cpool.tile([65, HWS], bf16, tag="xq2")
            nc.sync.dma_start(out=xq2[0:64, :], in_=xq2_d)
            nc.vector.memset(xq2[64:65, :], 1.0)
            xo1 = cpool.tile([64, HWS], bf16, tag="xo1")
            xo2 = cpool.tile([64, HWS], bf16, tag="xo2")

            def act_raw(out, in_, func, eng=None):
                eng = eng or nc.scalar
                ins = [eng.lower_ap(in_)]
                for v in (0.0, 1.0, 0.0):
                    ins.append(mybir.ImmediateValue(dtype=mybir.dt.float32, value=v))
                return eng.add_instruction(
                    mybir.InstActivation(
                        name=nc.get_next_instruction_name(), func=func,
                        ins=ins, outs=[eng.lower_ap(out)],
                    )
                )

            zb = cpool.tile([128, 1], f32, tag="zb")
            nc.vector.memset(zb[:, :], 0.0)
            epst = cpool.tile([1, 1], f32, tag="epst")
            nc.vector.memset(epst[:, :], EPS)

            eye = cp[:, 0:128]
            ref_yx = cp[0:2, 128:384]          # row0 = y, row1 = x
            ones1_128 = cp[0:1, 384:512]       # [1,128] ones (bcast lhsT)
            ones128_div = cp[0:128, 520:521]   # 1/64 on data rows, 0 on gaps

            BI = {0: (xq1, kvT0_d, xo1), 1: (xq2, kvT0_d, xo2), 2: (xq2, kvT1_d, xo2)}
            ST = [dict() for _ in range(NBLK)]   # per-block live tiles

            # ---------------- prefix stage 1: q projection ----------------
            def pre_q(blk):
                s = ST[blk]
                XQ, _, _ = BI[blk]
                pq_wT_sp = wq[:, blk * 128: (blk + 1) * 128]
                q_b = qpool.tile([128, HWS], bf16, tag="qb")
                for mc in range(8):
                    qp = tps.tile([128, 512], f32, tag="tl")
                    nc.tensor.matmul(
                        out=qp[:, :], lhsT=pq_wT_sp, rhs=XQ[:, mc * 512:(mc + 1) * 512],
                        start=True, stop=True,
                    )
                    act_raw(q_b[:, mc * 512:(mc + 1) * 512], qp[:, :], Act.Copy)
                s["q_b"] = q_b

            # ------------- prefix stage 2: offsets / indices ---------------
            def pre_off(blk):
                s = ST[blk]
                q_b = s["q_b"]
                bc0 = 527 + blk * 21
                dw_w = cp[:, bc0: bc0 + 16]
                dw_b = cp[:, bc0 + 16: bc0 + 17]
                ln_g = cp[:, bc0 + 17: bc0 + 18]
                ln_b = cp[:, bc0 + 18: bc0 + 19]
                pw_wT = cp[:, bc0 + 19: bc0 + 21]

                # depthwise 4x4 stride-4 conv (on bf16 q)
                q5 = q_b[:, :].rearrange("p (hh a ww b) -> p hh a ww b", hh=16, a=4, ww=16, b=4)
                asq = sbs.tile([128, 2, NS], f32, tag="asq")
                acc = asq[:, 0, :]
                sq = asq[:, 1, :]
                nc.vector.tensor_scalar(
                    out=acc, in0=q5[:, :, 0, :, 0], scalar1=dw_w[:, 0:1],
                    scalar2=None, op0=Alu.mult,
                )
                for t in range(1, 16):
                    dy, dx = t // 4, t % 4
                    nc.vector.scalar_tensor_tensor(
                        out=acc, in0=q5[:, :, dy, :, dx],
                        scalar=dw_w[:, t: t + 1], in1=acc,
                        op0=Alu.mult, op1=Alu.add,
                    )
                nc.vector.tensor_scalar(
                    out=acc, in0=acc, scalar1=dw_b, scalar2=None, op0=Alu.add
                )

                # layernorm over channels (mean via matmul with 1/64 weights)
                nc.vector.tensor_tensor(out=sq, in0=acc, in1=acc, op=Alu.mult)
                me_p = mps.tile([1, 2 * NS], f32, tag="m")
                nc.tensor.matmul(out=me_p[:, :], lhsT=ones128_div, rhs=asq[:, :, :],
                                 start=True, stop=True)
                stats = sbs.tile([1, 2 * NS], f32, tag="stats")
                nc.vector.tensor_copy(out=stats[:, 0:NS], in_=me_p[:, 0:NS])
                mu2 = sbs.tile([1, NS], f32, tag="mu2")
                nc.vector.tensor_tensor(out=mu2[:, :], in0=stats[:, 0:NS], in1=stats[:, 0:NS], op=Alu.mult)
                var = sbs.tile([1, NS], f32, tag="var")
                nc.vector.tensor_tensor(out=var[:, :], in0=me_p[:, NS:2 * NS], in1=mu2[:, :], op=Alu.subtract)
                sd = sbs.tile([1, NS], f32, tag="sd")
                nc.scalar.activation(out=sd[:, :], in_=var[:, :], func=Act.Sqrt, bias=epst[:, :])
                nc.vector.reciprocal(out=stats[:, NS:2 * NS], in_=sd[:, :])
                bc_p = mps.tile([128, 2 * NS], f32, tag="m")
                nc.tensor.matmul(out=bc_p[:, :], lhsT=ones1_128, rhs=stats[:, :], start=True, stop=True)
                t1 = sbs.tile([128, NS], f32, tag="t1")
                nc.vector.tensor_tensor(out=t1[:, :], in0=acc, in1=bc_p[:, 0:NS], op=Alu.subtract)
                nc.vector.tensor_tensor(out=t1[:, :], in0=t1[:, :], in1=bc_p[:, NS:2 * NS], op=Alu.mult)
                nc.vector.tensor_scalar(
                    out=t1[:, :], in0=t1[:, :], scalar1=ln_g, scalar2=ln_b,
                    op0=Alu.mult, op1=Alu.add,
                )
                gl = sbs.tile([128, NS], f32, tag="gl")
                nc.scalar.activation(out=gl[:, :], in_=t1[:, :], func=Act.Gelu, bias=zb[:, :])

                # offsets -> positions
                off_p = mps.tile([2, NS], f32, tag="m")
                nc.tensor.matmul(out=off_p[:, :], lhsT=pw_wT, rhs=gl[:, :], start=True, stop=True)
                pos = sbs.tile([2, NS], f32, tag="pos")
                nc.vector.tensor_tensor(out=pos[:, :], in0=off_p[:, :], in1=ref_yx, op=Alu.add)
                nc.vector.tensor_scalar(
                    out=pos[:, :], in0=pos[:, :], scalar1=1.0, scalar2=-1.0,
                    op0=Alu.min, op1=Alu.max,
                )

                # transpose pos -> [n,(y,x)] per 128-chunk
                posT = sbs.tile([128, 4], f32, tag="posT")  # cols: c0y c0x c1y c1x
                for c in range(2):
                    tp = mps.tile([128, 2], f32, tag="m")
                    nc.tensor.transpose(
                        out=tp[:, :], in_=pos[:, c * 128:(c + 1) * 128], identity=eye[0:2, 0:2]
                    )
                    nc.vector.tensor_copy(out=posT[:, c * 2: c * 2 + 2], in_=tp[:, :])

                # ---- index & weight math, batched over the two 128-chunks ----
                p2 = posT[:, :].rearrange("p (c two) -> p two c", two=2)
                y = p2[:, 0, :]   # [128, 2] strided
                x = p2[:, 1, :]
                idxkv = sbs.tile([128, 4], f32, tag="idxkv")
                idxw = sbs.tile([128, 8], f32, tag="idxw")
                fyb = bpool.tile([128, 2], f32, tag="fyb")
                wkv = sbs.tile([128, 8], f32, tag="wkv")   # w00 w01 w10 w11 per chunk
                dxw = sbs.tile([128, 4], f32, tag="dxw")   # (1-fxb, fxb) per chunk
                scr = sbs.tile([128, 24], f32, tag="scr")
                s2 = scr[:, :].rearrange("p (k c) -> p k c", c=2)
                xf, yf = s2[:, 0, :], s2[:, 1, :]
                xm, ym = s2[:, 2, :], s2[:, 3, :]
                x0, y0 = s2[:, 4, :], s2[:, 5, :]
                fx, fy = s2[:, 6, :], s2[:, 7, :]
                fx1, fy1 = s2[:, 8, :], s2[:, 9, :]
                ib, iw = s2[:, 10, :], s2[:, 11, :]

                # kv pixel coords
                nc.vector.tensor_scalar(out=xf, in0=x, scalar1=1.0, scalar2=31.5, op0=Alu.add, op1=Alu.mult)
                nc.vector.tensor_scalar(out=yf, in0=y, scalar1=1.0, scalar2=31.5, op0=Alu.add, op1=Alu.mult)
                # floor via round-to-nearest (+2^23) then fix-up (r > x)
                nc.vector.tensor_scalar(out=x0, in0=xf, scalar1=8388608.0, scalar2=-8388608.0, op0=Alu.add, op1=Alu.add)
                nc.vector.tensor_tensor(out=xm, in0=x0, in1=xf, op=Alu.is_gt)
                nc.vector.tensor_tensor(out=x0, in0=x0, in1=xm, op=Alu.subtract)
                nc.vector.tensor_scalar(out=x0, in0=x0, scalar1=62.0, scalar2=None, op0=Alu.min)
                nc.vector.tensor_scalar(out=y0, in0=yf, scalar1=8388608.0, scalar2=-8388608.0, op0=Alu.add, op1=Alu.add)
                nc.vector.tensor_tensor(out=ym, in0=y0, in1=yf, op=Alu.is_gt)
                nc.vector.tensor_tensor(out=y0, in0=y0, in1=ym, op=Alu.subtract)
                nc.vector.tensor_scalar(out=y0, in0=y0, scalar1=62.0, scalar2=None, op0=Alu.min)
                nc.vector.tensor_tensor(out=fx, in0=xf, in1=x0, op=Alu.subtract)
                nc.vector.tensor_tensor(out=fy, in0=yf, in1=y0, op=Alu.subtract)
                nc.vector.tensor_scalar(out=fx1, in0=fx, scalar1=-1.0, scalar2=1.0, op0=Alu.mult, op1=Alu.add)
                nc.vector.tensor_scalar(out=fy1, in0=fy, scalar1=-1.0, scalar2=1.0, op0=Alu.mult, op1=Alu.add)
                w4 = wkv[:, :].rearrange("p (c t) -> p t c", t=4)
                nc.vector.tensor_tensor(out=w4[:, 0, :], in0=fy1, in1=fx1, op=Alu.mult)
                nc.vector.tensor_tensor(out=w4[:, 1, :], in0=fy1, in1=fx, op=Alu.mult)
                nc.vector.tensor_tensor(out=w4[:, 2, :], in0=fy, in1=fx1, op=Alu.mult)
                nc.vector.tensor_tensor(out=w4[:, 3, :], in0=fy, in1=fx, op=Alu.mult)
                # kv gather indices: y0*64+x0 (+0,+1,+64,+65)
                nc.vector.scalar_tensor_tensor(out=ib, in0=y0, scalar=64.0, in1=x0, op0=Alu.mult, op1=Alu.add)
                i4 = idxkv[:, :].rearrange("p (c t) -> p t c", t=2)
                for t, offt in enumerate((0.0, 64.0)):
                    nc.vector.tensor_scalar(
                        out=i4[:, t, :], in0=ib, scalar1=offt, scalar2=None, op0=Alu.add,
                    )
                # bias window coords: cx = 31.5*(1-x), cy = 31.5*(1-y)
                nc.vector.tensor_scalar(out=xf, in0=x, scalar1=-31.5, scalar2=31.5, op0=Alu.mult, op1=Alu.add)
                nc.vector.tensor_scalar(out=yf, in0=y, scalar1=-31.5, scalar2=31.5, op0=Alu.mult, op1=Alu.add)
                nc.vector.tensor_scalar(out=x0, in0=xf, scalar1=8388608.0, scalar2=-8388608.0, op0=Alu.add, op1=Alu.add)
                nc.vector.tensor_tensor(out=xm, in0=x0, in1=xf, op=Alu.is_gt)
                nc.vector.tensor_tensor(out=x0, in0=x0, in1=xm, op=Alu.subtract)
                nc.vector.tensor_scalar(out=y0, in0=yf, scalar1=8388608.0, scalar2=-8388608.0, op0=Alu.add, op1=Alu.add)
                nc.vector.tensor_tensor(out=ym, in0=y0, in1=yf, op=Alu.is_gt)
                nc.vector.tensor_tensor(out=y0, in0=y0, in1=ym, op=Alu.subtract)
                nc.vector.tensor_tensor(out=fx, in0=xf, in1=x0, op=Alu.subtract)
                nc.vector.tensor_tensor(out=fyb[:, :], in0=yf, in1=y0, op=Alu.subtract)
                d2 = dxw[:, :].rearrange("p (c two) -> p two c", two=2)
                nc.vector.tensor_scalar(out=d2[:, 0, :], in0=fx, scalar1=-1.0, scalar2=1.0, op0=Alu.mult, op1=Alu.add)
                nc.vector.tensor_copy(out=d2[:, 1, :], in_=fx)
                # window index: ((x0b*128)+y0b)*65 + blk_base (+h stride)
                nc.vector.scalar_tensor_tensor(out=iw, in0=x0, scalar=128.0, in1=y0, op0=Alu.mult, op1=Alu.add)
                nc.vector.tensor_scalar(
                    out=iw, in0=iw, scalar1=130.0, scalar2=float(blk * TBLK),
                    op0=Alu.mult, op1=Alu.add,
                )
                iw4 = idxw[:, :].rearrange("p (c t) -> p t c", t=4)
                for hh in range(4):
                    nc.vector.tensor_scalar(
                        out=iw4[:, hh, :], in0=iw,
                        scalar1=float(hh * THEAD), scalar2=None, op0=Alu.add,
                    )

                idxkv_i = sbs.tile([128, 4], i32, tag="idxkvi")
                nc.vector.tensor_copy(out=idxkv_i[:, :], in_=idxkv[:, :])
                idxw_i = bpool.tile([128, 8], i32, tag="idxwi")
                nc.vector.tensor_copy(out=idxw_i[:, :], in_=idxw[:, :])

                # diag weight matrices for the two x-taps, per chunk
                diags = []
                for c in range(2):
                    d0 = bpool.tile([128, 128], bf16, tag=f"d0_{c}")
                    d1 = bpool.tile([128, 128], bf16, tag=f"d1_{c}")
                    nc.vector.tensor_scalar(out=d0[:, :], in0=eye, scalar1=dxw[:, c * 2: c * 2 + 1], scalar2=None, op0=Alu.mult)
                    nc.vector.tensor_scalar(out=d1[:, :], in0=eye, scalar1=dxw[:, c * 2 + 1: c * 2 + 2], scalar2=None, op0=Alu.mult)
                    diags.append((d0, d1))
                s.update(idxkv_i=idxkv_i, idxw_i=idxw_i, fyb=fyb, wkv=wkv, diags=diags)

            # ------------- prefix stage 3: kv gather + k/v proj -------------
            def pre_kv(blk):
                s = ST[blk]
                _, kvT_ap, _ = BI[blk]
                idxkv_i, wkv = s["idxkv_i"], s["wkv"]
                pk_wTs1 = wpb[0:65, blk * 192: blk * 192 + 128]
                pv_wT1 = wpb[0:65, blk * 192 + 128: blk * 192 + 192]

                G = sb.tile([128, 4, 128], f32, tag="G")
                for j in range(4):
                    nc.gpsimd.indirect_dma_start(
                        out=G[:, j, :], out_offset=None, in_=kvT_ap,
                        in_offset=IndirectOffsetOnAxis(ap=idxkv_i[:, j: j + 1], axis=0),
                    )
                xs_b = sbs.tile([65, NS], bf16, tag="xsb")
                nc.vector.memset(xs_b[64:65, :], 1.0)
                for c in range(2):
                    xsT = sbs.tile([128, 64], f32, tag="xsT")
                    nc.vector.tensor_scalar(
                        out=xsT[:, :], in0=G[:, c * 2, 0:64],
                        scalar1=wkv[:, c * 4: c * 4 + 1], scalar2=None, op0=Alu.mult,
                    )
                    for t in range(1, 4):
                        gsl = G[:, c * 2 + t // 2, (t % 2) * 64:(t % 2) * 64 + 64]
                        nc.vector.scalar_tensor_tensor(
                            out=xsT[:, :], in0=gsl,
                            scalar=wkv[:, c * 4 + t: c * 4 + t + 1], in1=xsT[:, :],
                            op0=Alu.mult, op1=Alu.add,
                        )
                    xs_p = mps.tile([64, 128], f32, tag="m")
                    nc.tensor.transpose(out=xs_p[:, :], in_=xsT[:, :], identity=eye)
                    nc.vector.tensor_copy(out=xs_b[0:64, c * 128:(c + 1) * 128], in_=xs_p[:, :])

                k_p = mps.tile([128, NS], f32, tag="m")
                nc.tensor.matmul(out=k_p[:, :], lhsT=pk_wTs1, rhs=xs_b[:, :], start=True, stop=True)
                k_b = bpool.tile([128, NS], bf16, tag="kb")
                nc.vector.tensor_copy(out=k_b[:, :], in_=k_p[:, :])

                vT1 = bpool.tile([128, 2, 128], bf16, tag="vT1")
                nc.vector.memset(vT1[:, :, :], 0.0)
                nc.vector.memset(vT1[:, :, :].rearrange("p c (h q) -> p c h q", q=32)[:, :, :, 16:17], 1.0)
                for c in range(2):
                    v_p = mps.tile([128, 64], f32, tag="m")
                    nc.tensor.matmul(
                        out=v_p[:, :], lhsT=xs_b[:, c * 128:(c + 1) * 128], rhs=pv_wT1,
                        start=True, stop=True,
                    )
                    vv = vT1[:, c, :].rearrange("p (h q) -> p h q", q=32)
                    nc.vector.tensor_copy(
                        out=vv[:, :, 0:16],
                        in_=v_p[:, :].rearrange("p (h q) -> p h q", q=16),
                    )
                s.update(k_b=k_b, vT1=vT1)

            # ------------- prefix stage 4: window gathers (prefetch) --------
            def pre_win(blk, heads):
                s = ST[blk]
                idxw_i = s["idxw_i"]
                slabs = s.setdefault("slabs", {})
                for h in heads:
                    for c in range(2):
                        Sl = spool.tile([128, 8320], bf16, tag="S")
                        nc.gpsimd.indirect_dma_start(
                            out=Sl[:, :], out_offset=None, in_=tab_d,
                            in_offset=IndirectOffsetOnAxis(ap=idxw_i[:, c * 4 + h: c * 4 + h + 1], axis=0),
                        )
                        slabs[(h, c)] = Sl

            # ------- y-lerp blend for one head (emitted one head ahead) ------
            def blend(blk, h):
                s = ST[blk]
                fyb = s["fyb"]
                ys = s.setdefault("Y", {})
                for c in range(2):
                    Sl = s["slabs"].pop((h, c))
                    S3 = Sl[:, :].rearrange("p (r two q) -> p r two q", two=2, q=65)
                    Yt = ypool.tile([128, 4160], bf16, tag="Y")
                    Y2 = Yt[:, :].rearrange("p (r q) -> p r q", q=65)
                    nc.vector.scalar_tensor_tensor(
                        out=Y2[:, :, :], in0=S3[:, :, 1, :], scalar=fyb[:, c: c + 1],
                        in1=S3[:, :, 0, :], op0=Alu.mult, op1=Alu.add,
                    )
                    ys[(h, c)] = Yt

            # ---------------- attention for one head ----------------
            def attn_head(blk, h):
                s = ST[blk]
                q_b, k_b, vT1, diags = s["q_b"], s["k_b"], s["vT1"], s["diags"]
                if h == 0:
                    avs_t = apool.tile([128, HWS], bf16, tag="avs")
                    s["avs"] = avs_t
                # emit next head's blend first so DVE runs it under our matmuls
                if h < NH - 1:
                    blend(blk, h + 1)
                elif blk + 1 < NBLK:
                    blend(blk + 1, 0)
                avs = s["avs"]
                P = ppool.tile([128, 2, HWS], bf16, tag="P")
                for c in range(2):
                    Yt = s["Y"].pop((h, c))
                    Y3 = Yt[:, :].rearrange("p (r q) -> p r q", q=65)
                    d0, d1 = diags[c]
                    kh = k_b[h * 32: h * 32 + 16, c * 128:(c + 1) * 128]
                    for wv in range(4):
                        pts = []
                        for i in range(2):
                            mc = wv * 2 + i
                            pt = qkps.tile([128, 512], f32, tag="qkp")
                            nc.tensor.matmul(
                                out=pt[:, :], lhsT=kh,
                                rhs=q_b[h * 32: h * 32 + 16, mc * 512:(mc + 1) * 512],
                                start=True, stop=False, tile_position=(h * 32, 0),
                            )
                            pts.append(pt)
                        for i in range(2):
                            mc = wv * 2 + i
                            nc.tensor.matmul(
                                out=pts[i][:, :], lhsT=d0,
                                rhs=Y3[:, mc * 8:(mc + 1) * 8, 0:64],
                                start=False, stop=False,
                            )
                        for i in range(2):
                            mc = wv * 2 + i
                            nc.tensor.matmul(
                                out=pts[i][:, :], lhsT=d1,
                                rhs=Y3[:, mc * 8:(mc + 1) * 8, 1:65],
                                start=False, stop=True,
                            )
                        for i in range(2):
                            mc = wv * 2 + i
                            nc.scalar.activation(
                                out=P[:, c, mc * 512:(mc + 1) * 512], in_=pts[i][:, :],
                                func=Act.Exp, bias=zb[:, :],
                            )
                # AV for this head: [32,512] psum (row 16 = sums), then spread copy
                for pr in range(4):
                    mca, mcb = pr * 2, pr * 2 + 1
                    a0 = avps.tile([32, 512], f32, tag="avp")
                    a1 = avps.tile([32, 512], f32, tag="avp")
                    for c in range(2):
                        lw = vT1[:, c, h * 32:(h + 1) * 32]
                        nc.tensor.matmul(out=a0[:, :], lhsT=lw, rhs=P[:, c, mca * 512:(mca + 1) * 512], start=(c == 0), stop=(c == 1))
                        nc.tensor.matmul(out=a1[:, :], lhsT=lw, rhs=P[:, c, mcb * 512:(mcb + 1) * 512], start=(c == 0), stop=(c == 1))
                    act_raw(avs[h * 32:(h + 1) * 32, mca * 512:(mca + 1) * 512], a0[:, :], Act.Copy)
                    nc.vector.tensor_copy(out=avs[h * 32:(h + 1) * 32, mcb * 512:(mcb + 1) * 512], in_=a1[:, :])

            # ---------------- tail: normalize + out proj + residual --------
            def tail(blk):
                s = ST[blk]
                avs = s["avs"]
                _, _, R = BI[blk]
                po_wT_sp = cpb[:, 128 + blk * 64: 128 + (blk + 1) * 64]
                b4 = cpb[:, 0:128]
                po_b = cp[0:64, 524 + blk: 525 + blk]
                onf = apool.tile([128, HWS], bf16, tag="onf")
                for mc in range(8):
                    sb_p = tps.tile([128, 512], f32, tag="tl")
                    nc.tensor.matmul(out=sb_p[:, :], lhsT=b4, rhs=avs[:, mc * 512:(mc + 1) * 512], start=True, stop=True)
                    rcp = sbs.tile([128, 512], f32, tag="rcp")
                    act_raw(rcp[:, :], sb_p[:, :], Act.Reciprocal)
                    nc.vector.tensor_tensor(out=onf[:, mc * 512:(mc + 1) * 512], in0=avs[:, mc * 512:(mc + 1) * 512], in1=rcp[:, :], op=Alu.mult)
                for mc in range(8):
                    op = mps.tile([64, 512], f32, tag="m")
                    nc.tensor.matmul(out=op[:, :], lhsT=po_wT_sp, rhs=onf[:, mc * 512:(mc + 1) * 512], start=True, stop=True)
                    if blk == 2:
                        nc.vector.scalar_tensor_tensor(
                            out=R[:, mc * 512:(mc + 1) * 512], in0=op[:, :], scalar=po_b,
                            in1=R[:, mc * 512:(mc + 1) * 512], op0=Alu.add, op1=Alu.add,
                        )
                    else:
                        nc.vector.tensor_scalar(
                            out=R[:, mc * 512:(mc + 1) * 512], in0=op[:, :],
                            scalar1=po_b, scalar2=None, op0=Alu.add,
                        )

            # =================== pipelined emission ===================
            pre_q(0)
            pre_q(1)
            pre_q(2)
            pre_off(0)
            pre_kv(0)
            pre_win(0, (0, 1))
            blend(0, 0)
            for blk in range(NBLK):
                nxt = blk + 1
                for h in range(NH):
                    if nxt < NBLK:
                        if h == 2:
                            pre_off(nxt)
                        elif h == 3:
                            pre_kv(nxt)
                            pre_win(nxt, (0, 1))
                    attn_head(blk, h)
                    if h == 0:
                        pre_win(blk, (2, 3))
                tail(blk)
                if blk == 0:
                    nc.sync.dma_start(out=o1_d, in_=xo1[:, :])
            nc.sync.dma_start(out=o2_d, in_=xo2[:, :])

    nc.compile()
    return nc


def _host_prep(inputs):
    """Build per-core in_maps. inputs: dict of full numpy arrays."""
    import ml_dtypes

    x0, x1, x2 = inputs["x0"], inputs["x1"], inputs["x2"]

    def spread_cols(m):
        # m: [64(in), 64(out)] -> [64(in), 128] with out col h*16+j at h*32+j
        out = np.zeros((m.shape[0], 128), m.dtype)
        for h in range(4):
            out[:, h * 32: h * 32 + 16] = m[:, h * 16: (h + 1) * 16]
        return out

    def spread_rows(v):
        # v: [64, k] -> [128, k] with row h*16+j at h*32+j
        out = np.zeros((128,) + v.shape[1:], v.dtype)
        for h in range(4):
            out[h * 32: h * 32 + 16] = v[h * 16: (h + 1) * 16]
        return out

    # q-projection weights bf16: [65, 3*128]  (spread pq_wT, bias row 64)
    wq = np.zeros((65, 3 * 128), ml_dtypes.bfloat16)
    for b in range(3):
        wq[0:64, b * 128: (b + 1) * 128] = spread_cols(inputs["pq_w"][b].T).astype(
            ml_dtypes.bfloat16
        )
        wq[64, b * 128: (b + 1) * 128] = spread_rows(
            inputs["pq_b"][b][:, None]
        )[:, 0].astype(ml_dtypes.bfloat16)
    wpb = np.zeros((65, 3 * 192), ml_dtypes.bfloat16)
    for b in range(3):
        o = b * 192
        pk = np.zeros((65, 128), np.float32)
        pk[0:64] = spread_cols(inputs["pk_w"][b].T * 0.25)
        for h in range(4):
            pk[64, h * 32: h * 32 + 16] = inputs["pk_b"][b][h * 16: (h + 1) * 16] * 0.25
        wpb[:, o: o + 128] = pk.astype(ml_dtypes.bfloat16)
        wpb[:64, o + 128: o + 192] = inputs["pv_w"][b].T.astype(ml_dtypes.bfloat16)
        wpb[64, o + 128: o + 192] = inputs["pv_b"][b].astype(ml_dtypes.bfloat16)
    # const pack [128, 590]
    cp = np.zeros((128, 590), np.float32)
    cp[:, 0:128] = np.eye(128, dtype=np.float32)
    ys = (np.linspace(0.5, HK - 0.5, HK) / (HK - 1.0)) * 2.0 - 1.0
    cp[0, 128:384] = np.repeat(ys, WK)         # y per n (i-major)
    cp[1, 128:384] = np.tile(ys, HK)           # x per n
    cp[0, 384:512] = 1.0                       # ones1_128
    for h in range(4):
        cp[h * 32: h * 32 + 16, 520] = 1.0 / 64.0
    for b in range(3):
        cp[:, 521 + b] = spread_rows(inputs["pq_b"][b][:, None])[:, 0]
        cp[64:128, 524 + b] = inputs["po_b"][b]
        cp[0:64, 524 + b] = inputs["po_b"][b]
        bc0 = 527 + b * 21
        cp[:, bc0: bc0 + 16] = spread_rows(inputs["dw_w"][b].reshape(64, 16))
        cp[:, bc0 + 16] = spread_rows(inputs["dw_b"][b][:, None])[:, 0]
        cp[:, bc0 + 17] = spread_rows(inputs["ln_g"][b][:, None])[:, 0]
        cp[:, bc0 + 18] = spread_rows(inputs["ln_b"][b][:, None])[:, 0]
        cp[:, bc0 + 19: bc0 + 21] = spread_rows(inputs["pw_w"][b].T)
    cpb = np.zeros((128, 320), ml_dtypes.bfloat16)
    b4 = np.zeros((128, 128), np.float32)
    for h in range(4):
        b4[h * 32 + 16, h * 32: (h + 1) * 32] = 1.0
    cpb[:, 0:128] = b4.astype(ml_dtypes.bfloat16)
    for b in range(3):
        poT = inputs["po_w"][b].T  # [c, o]
        for h in range(4):
            cpb[h * 32: h * 32 + 16, 128 + b * 64: 128 + (b + 1) * 64] = poT[
                h * 16: (h + 1) * 16
            ].astype(ml_dtypes.bfloat16)
    # rpe slice tables bf16, row-interleaved: [b,h,x0,row,(W,D),col]
    tab = np.zeros((NBLK, NH, 64, TROW, 2, TCOL), ml_dtypes.bfloat16)
    rpe = inputs["rpe"]
    for b in range(3):
        for h in range(4):
            pad = np.zeros((129, 128), np.float32)
            pad[0:127, 0:127] = rpe[b, h]
            dif = pad[1:129] - pad[0:128]
            for x0s in range(64):
                tab[b, h, x0s, :, 0, :] = pad[0:128, x0s: x0s + 65].astype(ml_dtypes.bfloat16)
                tab[b, h, x0s, :, 1, :] = dif[:, x0s: x0s + 65].astype(ml_dtypes.bfloat16)
    tab = tab.reshape(-1, 1)

    in_maps = []
    for bb in range(B):
        m = {
            "xq1": np.ascontiguousarray(x1[bb, :64].reshape(64, HWS)).astype(ml_dtypes.bfloat16),
            "xq2": np.ascontiguousarray(x2[bb, :64].reshape(64, HWS)).astype(ml_dtypes.bfloat16),
            "kvT0": np.ascontiguousarray(x0[bb, :64].reshape(64, HWS).T),
            "kvT1": np.ascontiguousarray(x1[bb, :64].reshape(64, HWS).T),
            "wq": wq,
            "wpb": wpb,
            "cp": cp,
            "cpb": cpb,
            "rpetab": tab,
        }
        in_maps.append(m)
    return in_maps


def kernel(**inputs):
    from concourse.bass_utils import run_bass_kernel_spmd

    if "nc" not in _CACHE:
        _CACHE["nc"] = _build_graph()
    nc = _CACHE["nc"]
    in_maps = _host_prep(inputs)
    res = run_bass_kernel_spmd(nc, in_maps, core_ids=list(range(8)))
    out = np.zeros((NBLK, B, C, H, W), np.float32)
    out[0] = inputs["x0"]
    out[1, :, :64] = inputs["x1"][:, :64]
    out[2, :, :64] = inputs["x2"][:, :64]
    for bb in range(B):
        out[1, bb, 64:] = inputs["x1"][bb, 64:] + res.results[bb]["o1"].astype(np.float32).reshape(64, H, W)
        out[2, bb, 64:] = inputs["x2"][bb, 64:] + res.results[bb]["o2"].astype(np.float32).reshape(64, H, W)
    return out
